# revision 6
# baseline (speedup 1.0000x reference)
"""GAT (2-layer) on 8 Trainium2 NeuronCores — edge-parallel by destination.

Strategy
--------
- Nodes are sharded 8 ways (6250/core). Edges (incl. self-loops) are routed to
  the core that owns their destination node, so each core's scatter-adds are
  purely local (no all-reduce of aggregates).
- Phase A (per core): GEMM over its node shard computes h1 plus the attention
  logit projections a_src/a_dst (folded into the GEMM as extra output columns);
  rows [h1|a_src] are packed fp16 into a 768B-stride table, AllGathered so every
  core holds the full table.
- Edge phase (per core): edges are grouped by (128-node dst block, src-half) and
  chunked 128 at a time. Per chunk: dma_gather of the 128 source rows; a one-hot
  (edge x node) matrix built by is_equal against an iota constant; a_dst
  expanded edge-wise via a small matmul with the transposed one-hot; logits ->
  LeakyReLU(0.2) -> exp on the scalar engine; messages = h1_src * exp; the
  weighted scatter-sum is one fp16 matmul accumulating [128 nodes x (msg|exp)]
  in PSUM over the block (softmax denominator rides along as an extra column).
  Normalization (divide by the exp-sum) happens once per node at the end
  (softmax is shift-invariant; logits are O(1) so no max-subtraction needed).
- Between layers: ELU, second GEMM (x @ W2ext), second table, AllGather, same
  edge phase with 1 head, then the final normalize produces the output shard.
"""

import math
import os

import numpy as np

import concourse.bass as bass
import concourse.mybir as mybir
import concourse.tile as tile
from concourse import bacc
from concourse.bass_utils import run_bass_kernel_spmd
from concourse.masks import make_identity

# problem constants (from the reference)
N = 50000
E = 500000
EMB = 128
HIDDEN = 256
HEADS = 8
OUT1 = 32
REPR = 64
NEG_SLOPE = 0.2

NC = 8
P = 128
NSH = N // NC                    # 6250 nodes per core
NBLK = (NSH + P - 1) // P        # 49 dst blocks per core
LASTB = NSH - (NBLK - 1) * P     # 106 nodes in last block
HALF = N // 2                    # src table split (int16 gather indices)

T1_ELEM = 384                    # fp16: h1(256) | a_src(8) | pad -> 768B rows
T2_ELEM = 256                    # fp16: h2(64) | a2_src(1) | pad -> 512B rows

F16 = mybir.dt.float16
F32 = mybir.dt.float32
I16 = mybir.dt.int16
AF = mybir.ActivationFunctionType
ALU = mybir.AluOpType

MAX_CH_PER_CALL = 7              # 896 idxs/call, under the 1024-desc SWDGE ring


def _prep_edges(edge_index):
    """Partition + sort edges; build per-core gather-index / dst-local arrays."""
    ei = np.asarray(edge_index)
    src = np.concatenate([ei[0], np.arange(N, dtype=np.int64)]).astype(np.int64)
    dst = np.concatenate([ei[1], np.arange(N, dtype=np.int64)]).astype(np.int64)

    core = dst // NSH
    per_core = []
    kch = 1
    for c in range(NC):
        m = core == c
        s, d = src[m], dst[m] - c * NSH
        blk = d >> 7
        half = (s >= HALF).astype(np.int64)
        key = blk * 2 + half
        order = np.argsort(key, kind="stable")
        s, d, key = s[order], d[order], key[order]
        counts = np.bincount(key, minlength=NBLK * 2)
        kch = max(kch, int(math.ceil(counts.max() / P)))
        per_core.append((s, d, key, counts))

    nchunk = NBLK * 2 * kch
    idx_all = np.zeros((NC, NBLK * 2, kch * P), np.int16)
    dl_all = np.full((NC, NBLK * 2, kch * P), 200.0, np.float16)
    for c in range(NC):
        s, d, key, counts = per_core[c]
        starts = np.zeros(NBLK * 2 + 1, np.int64)
        np.cumsum(counts, out=starts[1:])
        for g in range(NBLK * 2):
            n = counts[g]
            if n == 0:
                continue
            sl = slice(starts[g], starts[g] + n)
            h = g & 1
            idx_all[c, g, :n] = (s[sl] - h * HALF).astype(np.int16)
            dl_all[c, g, :n] = (d[sl] & 127).astype(np.float16)

    # wrap gather indices per call: idx i -> [i%16, i//16], replicated 8x to 128 rows
    ncalls_per_group = math.ceil(kch / MAX_CH_PER_CALL)
    call_ch = []                 # chunks per call, per group (same for all groups)
    left = kch
    for _ in range(ncalls_per_group):
        take = min(MAX_CH_PER_CALL, left)
        call_ch.append(take)
        left -= take
    wcols = kch * P // 16        # idx columns per group
    idx_w = np.zeros((NC, 128, NBLK * 2 * wcols), np.int16)
    for c in range(NC):
        for g in range(NBLK * 2):
            pos = 0
            for t in call_ch:
                n = t * P
                w = idx_all[c, g, pos:pos + n].reshape(n // 16, 16).T  # [16, n/16]
                col0 = g * wcols + pos // 16
                idx_w[c, :, col0:col0 + n // 16] = np.tile(w, (8, 1))
                pos += n
    # dst-local per chunk, edge i of chunk -> partition i: [128, nchunk]
    dl_T = np.ascontiguousarray(
        dl_all.reshape(NC, NBLK * 2 * kch, P).transpose(0, 2, 1))
    return kch, call_ch, idx_w, dl_T


def _build(kch, call_ch, b1_any, b2_any):
    nchunk = NBLK * 2 * kch
    cpb = 2 * kch                # chunks per dst block
    wcols = kch * P // 16
    nc = bacc.Bacc(None, target_bir_lowering=False)

    # ---- inputs (per core) ----
    embT_in = nc.dram_tensor("embT", [EMB, NBLK * P], F16, kind="ExternalInput")
    w1_in = nc.dram_tensor("w1ext", [EMB, HIDDEN + 16], F16, kind="ExternalInput")
    w2_in = nc.dram_tensor("w2ext", [HIDDEN, REPR + 2], F16, kind="ExternalInput")
    idx_in = nc.dram_tensor("idxw", [128, NBLK * 2 * wcols], I16, kind="ExternalInput")
    dl_in = nc.dram_tensor("dlT", [128, nchunk], F16, kind="ExternalInput")
    iota_in = nc.dram_tensor("iota", [P, P], F16, kind="ExternalInput")
    iotar_in = nc.dram_tensor("iotar", [P, cpb * P], F16, kind="ExternalInput")
    if b1_any:
        b1_in = nc.dram_tensor("b1e", [P, HIDDEN], F32, kind="ExternalInput")
    if b2_any:
        b2_in = nc.dram_tensor("b2e", [P, REPR], F32, kind="ExternalInput")
    out_t = nc.dram_tensor("out", [NSH, REPR], F32, kind="ExternalOutput")

    # ---- internal DRAM ----
    t1_shard = nc.dram_tensor("t1_shard", [NSH, T1_ELEM], F16, kind="Internal")
    t1_full = nc.dram_tensor("t1_full", [N, T1_ELEM], F16, kind="Internal",
                             addr_space="Shared")
    t2_shard = nc.dram_tensor("t2_shard", [NSH, T2_ELEM], F16, kind="Internal")
    t2_full = nc.dram_tensor("t2_full", [N, T2_ELEM], F16, kind="Internal",
                             addr_space="Shared")

    with tile.TileContext(nc) as tc:
        with (
            tc.tile_pool(name="const", bufs=1) as cst,
            tc.tile_pool(name="sb", bufs=3) as sb,
            tc.tile_pool(name="gp", bufs=4) as gp,
            tc.tile_pool(name="rp", bufs=4) as rp,
            tc.tile_pool(name="sb2", bufs=3) as sb2,
            tc.tile_pool(name="oh", bufs=4) as ohp,
            tc.tile_pool(name="psA", bufs=3, space="PSUM") as psA,
            tc.tile_pool(name="psB", bufs=2, space="PSUM") as psB,
            tc.tile_pool(name="psC", bufs=2, space="PSUM") as psC,
            tc.tile_pool(name="psD", bufs=1, space="PSUM") as psD,
        ):
            # ---- constants ----
            iota = cst.tile([P, P], F16)
            nc.sync.dma_start(out=iota[:], in_=iota_in[:])
            iotar = cst.tile([P, cpb * P], F16)
            nc.sync.dma_start(out=iotar[:], in_=iotar_in[:])
            ident = cst.tile([P, P], F16)
            make_identity(nc, ident[:])
            w1 = cst.tile([EMB, HIDDEN + 16], F16)
            nc.sync.dma_start(out=w1[:], in_=w1_in[:])
            w2 = cst.tile([P, 2, REPR + 2], F16)
            nc.sync.dma_start(out=w2[:, 0, :], in_=w2_in[0:P, :])
            nc.sync.dma_start(out=w2[:, 1, :], in_=w2_in[P:HIDDEN, :])
            it_all = cst.tile([128, NBLK * 2 * wcols], I16)
            nc.sync.dma_start(out=it_all[:], in_=idx_in[:])
            dl_all = cst.tile([128, nchunk], F16)
            nc.sync.dma_start(out=dl_all[:], in_=dl_in[:])
            adst1 = cst.tile([P, NBLK * 8], F16)
            adst2 = cst.tile([P, NBLK], F16)
            if b1_any:
                b1e = cst.tile([P, HIDDEN], F32)
                nc.sync.dma_start(out=b1e[:], in_=b1_in[:])
            if b2_any:
                b2e = cst.tile([P, REPR], F32)
                nc.sync.dma_start(out=b2e[:], in_=b2_in[:])

            # ---- phase A: h1 GEMM + table build (4 blocks per DMA) ----
            GA = 4
            for b0 in range(0, NBLK, GA):
                nb = min(GA, NBLK - b0)
                et = sb.tile([EMB, GA, P], F16, tag="embT")
                nc.sync.dma_start(
                    out=et[:, 0:nb, :].rearrange("p a n -> p (a n)"),
                    in_=embT_in[:, b0 * P:(b0 + nb) * P])
                t1s = sb.tile([P, GA, T1_ELEM], F16, tag="t1s")
                for j in range(nb):
                    b = b0 + j
                    ph1 = psA.tile([P, HIDDEN + 16], F32, tag="acc")
                    nc.tensor.matmul(out=ph1[:], lhsT=et[:, j, :], rhs=w1[:],
                                     start=True, stop=True)
                    nc.scalar.copy(out=t1s[:, j, 0:HIDDEN], in_=ph1[:, 0:HIDDEN])
                    nc.scalar.copy(out=t1s[:, j, HIDDEN:HIDDEN + 8],
                                   in_=ph1[:, HIDDEN:HIDDEN + 8])
                    nc.vector.tensor_copy(out=adst1[:, b * 8:(b + 1) * 8],
                                          in_=ph1[:, HIDDEN + 8:HIDDEN + 16])
                full = nb if b0 + nb < NBLK else nb - 1
                if full:
                    nc.sync.dma_start(
                        out=t1_shard[b0 * P:(b0 + full) * P, :]
                            .rearrange("(a p) e -> p a e", p=P),
                        in_=t1s[:, 0:full, :])
                if full < nb:
                    nc.sync.dma_start(
                        out=t1_shard[(b0 + full) * P:(b0 + full) * P + LASTB, :],
                        in_=t1s[:LASTB, full, :])

            nc.gpsimd.collective_compute(
                "AllGather", ALU.bypass, ins=[t1_shard[:]], outs=[t1_full[:]],
                replica_groups=[list(range(NC))])

            # ---- edge phase helper ----
            def edge_layer(t_full, elem, hid, heads, adst_t, out_cb):
                """One GAT message-passing layer over this core's dst blocks.

                t_full: gather table [N, elem] fp16, row = [feat(hid)|a_src(heads)|pad]
                adst_t: [P, NBLK*heads] per-block a_dst values
                out_cb(b, ps_acc): consume the accumulated [P, hid+heads] psum
                """
                mcols = hid + heads          # matmul rhs columns (msg | exp)
                for b in range(NBLK):
                    gs = []                  # gathered tiles per call
                    for h in (0, 1):
                        g_idx = b * 2 + h
                        pos = 0
                        for t in call_ch:
                            gt = gp.tile([P, MAX_CH_PER_CALL, elem], F16,
                                         tag=f"g{hid}")
                            col0 = g_idx * wcols + pos * 8
                            nc.gpsimd.dma_gather(
                                out_ap=gt[:, 0:t, :],
                                in_ap=t_full[h * HALF:(h + 1) * HALF, :],
                                idxs_ap=it_all[:, col0:col0 + t * 8],
                                num_idxs=t * P, num_idxs_reg=t * P,
                                elem_size=elem)
                            gs.append((gt, t))
                            pos += t
                    # one-hots (one batched is_equal) + transposed one-hots
                    pse = psC.tile([P, cpb * heads], F32, tag="adst")
                    oh_all = ohp.tile([P, cpb, P], F16, tag="oh")
                    nc.vector.tensor_tensor(
                        out=oh_all[:],
                        in0=dl_all[:, b * cpb:(b + 1) * cpb]
                            .rearrange("p (t o) -> p t o", o=1)
                            .to_broadcast([P, cpb, P]),
                        in1=iotar[:].rearrange("p (t n) -> p t n", n=P),
                        op=ALU.is_equal)
                    ohs = [oh_all[:, k, :] for k in range(cpb)]
                    ohT_sb = ohp.tile([P, cpb, P], F16, tag="ohT_sb")
                    PSB_CH = 8   # chunks per fp16 psum bank
                    for g0 in range(0, cpb, PSB_CH):
                        g1 = min(g0 + PSB_CH, cpb)
                        pst = psB.tile([P, PSB_CH, P], F16, tag="ohT")
                        for k in range(g0, g1):
                            nc.tensor.transpose(out=pst[:, k - g0, :], in_=ohs[k],
                                                identity=ident[:])
                        nc.scalar.copy(out=ohT_sb[:, g0:g1, :].rearrange("p t n -> p (t n)"),
                                       in_=pst[:, 0:g1 - g0, :].rearrange("p t n -> p (t n)"))
                    for k in range(cpb):
                        nc.tensor.matmul(
                            out=pse[:, k * heads:(k + 1) * heads],
                            lhsT=ohT_sb[:, k, :],
                            rhs=adst_t[:, b * heads:(b + 1) * heads],
                            start=True, stop=True)
                    # logits -> leaky -> exp  (batched over the block's chunks)
                    e_sb = sb2.tile([P, cpb * heads], F32, tag=f"e{hid}")
                    k = 0
                    for gt, t in gs:
                        nc.vector.tensor_tensor(
                            out=e_sb[:, k * heads:(k + t) * heads]
                                .rearrange("p (t h) -> p t h", t=t),
                            in0=gt[:, 0:t, hid:hid + heads],
                            in1=pse[:, k * heads:(k + t) * heads]
                                .rearrange("p (t h) -> p t h", t=t),
                            op=ALU.add)
                        k += t
                    lk = sb2.tile([P, cpb * heads], F32, tag=f"lk{hid}")
                    nc.scalar.activation(out=lk[:], in_=e_sb[:], func=AF.Prelu,
                                         alpha=NEG_SLOPE)
                    ex = sb2.tile([P, cpb * heads], F16, tag=f"ex{hid}")
                    nc.scalar.activation(out=ex[:], in_=lk[:], func=AF.Exp)
                    # messages (feat * exp, broadcast over feat/head) + exp col
                    rhs = rp.tile([P, cpb, mcols], F16, tag=f"rhs{hid}")
                    k = 0
                    for gt, t in gs:
                        nc.vector.tensor_tensor(
                            out=rhs[:, k:k + t, 0:hid]
                                .rearrange("p t (h d) -> p t h d", h=heads),
                            in0=gt[:, 0:t, 0:hid]
                                .rearrange("p t (h d) -> p t h d", h=heads),
                            in1=ex[:, k * heads:(k + t) * heads]
                                .rearrange("p (t h) -> p t h", t=t)[:, :, :, None]
                                .to_broadcast([P, t, heads, hid // heads]),
                            op=ALU.mult)
                        k += t
                    nc.vector.tensor_copy(
                        out=rhs[:, :, hid:hid + heads],
                        in_=ex[:].rearrange("p (t h) -> p t h", t=cpb))
                    # scatter-accumulate into the block's psum
                    pacc = psA.tile([P, mcols], F32, tag="acc")
                    for k in range(cpb):
                        nc.tensor.matmul(out=pacc[:], lhsT=ohs[k],
                                         rhs=rhs[:, k, :], start=(k == 0),
                                         stop=(k == cpb - 1))
                    out_cb(b, pacc)

            # ---- layer 1 block finisher: normalize, ELU, GEMM2, T2 rows ----
            def finish1(b, pacc):
                rows = P if b < NBLK - 1 else LASTB
                se = sb.tile([P, HEADS], F32, tag="se")
                nc.vector.tensor_scalar_add(out=se[:], in0=pacc[:, HIDDEN:HIDDEN + 8],
                                            scalar1=1e-16)
                rec = sb.tile([P, HEADS], F32, tag="rec")
                nc.vector.reciprocal(out=rec[:], in_=se[:])
                v = sb.tile([P, HIDDEN], F32, tag="v")
                nc.vector.tensor_tensor(
                    out=v[:].rearrange("p (h d) -> p h d", h=HEADS),
                    in0=pacc[:, 0:HIDDEN].rearrange("p (h d) -> p h d", h=HEADS),
                    in1=rec[:, :, None].to_broadcast([P, HEADS, OUT1]),
                    op=ALU.mult)
                if b1_any:
                    nc.vector.tensor_tensor(out=v[:], in0=v[:], in1=b1e[:], op=ALU.add)
                # elu(v) = relu(v) + exp(min(v,0)) - 1
                r = sb.tile([P, HIDDEN], F32, tag="relu")
                nc.scalar.activation(out=r[:], in_=v[:], func=AF.Relu)
                mn = sb.tile([P, HIDDEN], F32, tag="mn")
                nc.vector.tensor_scalar_min(out=mn[:], in0=v[:], scalar1=0.0)
                em = sb.tile([P, HIDDEN], F32, tag="em")
                nc.scalar.activation(out=em[:], in_=mn[:], func=AF.Exp)
                x = sb.tile([P, HIDDEN], F32, tag="x")
                nc.vector.tensor_tensor(out=x[:], in0=r[:], in1=em[:], op=ALU.add)
                x16 = sb.tile([P, HIDDEN], F16, tag="x16")
                nc.vector.tensor_scalar_add(out=x16[:], in0=x[:], scalar1=-1.0)
                # GEMM2: h2 = x @ W2ext  (transpose x tiles for lhsT)
                xT = sb.tile([P, 2, P], F16, tag="xT")
                for k in range(2):
                    pst = psD.tile([P, P], F16, tag="misc")
                    nc.tensor.transpose(out=pst[:], in_=x16[:, k * P:(k + 1) * P],
                                        identity=ident[:])
                    nc.scalar.copy(out=xT[:, k, :], in_=pst[:])
                ph2 = psD.tile([P, REPR + 2], F32, tag="misc")
                for k in range(2):
                    nc.tensor.matmul(out=ph2[:], lhsT=xT[:, k, :], rhs=w2[:, k, :],
                                     start=(k == 0), stop=(k == 1))
                t2s = sb.tile([P, T2_ELEM], F16, tag="t2s")
                nc.scalar.copy(out=t2s[:, 0:REPR], in_=ph2[:, 0:REPR])
                nc.vector.tensor_copy(out=t2s[:, REPR:REPR + 1],
                                      in_=ph2[:, REPR:REPR + 1])
                nc.vector.tensor_copy(out=adst2[:, b:b + 1],
                                      in_=ph2[:, REPR + 1:REPR + 2])
                nc.sync.dma_start(out=t2_shard[b * P:b * P + rows, :],
                                  in_=t2s[:rows, :])

            edge_layer(t1_full, T1_ELEM, HIDDEN, HEADS, adst1, finish1)

            nc.gpsimd.collective_compute(
                "AllGather", ALU.bypass, ins=[t2_shard[:]], outs=[t2_full[:]],
                replica_groups=[list(range(NC))])

            # ---- layer 2 block finisher: normalize -> output ----
            def finish2(b, pacc):
                rows = P if b < NBLK - 1 else LASTB
                se = sb.tile([P, 1], F32, tag="se2")
                nc.vector.tensor_scalar_add(out=se[:], in0=pacc[:, REPR:REPR + 1],
                                            scalar1=1e-16)
                rec = sb.tile([P, 1], F32, tag="rec2")
                nc.vector.reciprocal(out=rec[:], in_=se[:])
                o = sb.tile([P, REPR], F32, tag="o")
                nc.scalar.activation(out=o[:], in_=pacc[:, 0:REPR], func=AF.Copy,
                                     scale=rec[:, 0:1])
                if b2_any:
                    nc.vector.tensor_tensor(out=o[:], in0=o[:], in1=b2e[:], op=ALU.add)
                nc.sync.dma_start(out=out_t[b * P:b * P + rows, :], in_=o[:rows, :])

            edge_layer(t2_full, T2_ELEM, REPR, 1, adst2, finish2)

    nc.finalize()
    globals()["LAST_NC"] = nc
    return nc


def kernel(**inputs):
    node_emb = np.asarray(inputs["node_emb"], np.float32)
    W1 = np.asarray(inputs["W1"], np.float32)
    att1_src = np.asarray(inputs["att1_src"], np.float32)
    att1_dst = np.asarray(inputs["att1_dst"], np.float32)
    b1 = np.asarray(inputs["b1"], np.float32)
    W2 = np.asarray(inputs["W2"], np.float32)
    att2_src = np.asarray(inputs["att2_src"], np.float32)
    att2_dst = np.asarray(inputs["att2_dst"], np.float32)
    b2 = np.asarray(inputs["b2"], np.float32)
    edge_index = np.asarray(inputs["edge_index"])

    kch, call_ch, idx_w, dl_T = _prep_edges(edge_index)

    # fold attention projections into the GEMMs: a_src = emb @ (W1 . att)
    A1s = np.einsum("ehd,hd->eh", W1.reshape(EMB, HEADS, OUT1), att1_src)
    A1d = np.einsum("ehd,hd->eh", W1.reshape(EMB, HEADS, OUT1), att1_dst)
    w1ext = np.concatenate([W1, A1s, A1d], axis=1).astype(np.float16)
    A2s = W2 @ att2_src[0]
    A2d = W2 @ att2_dst[0]
    w2ext = np.concatenate([W2, A2s[:, None], A2d[:, None]], axis=1).astype(np.float16)

    iota = np.tile(np.arange(P, dtype=np.float16), (P, 1))
    iotar = np.tile(np.arange(P, dtype=np.float16), (P, 2 * kch))
    b1_any = bool(np.any(b1))
    b2_any = bool(np.any(b2))

    nc = _build(kch, call_ch, b1_any, b2_any)

    embT_pad = np.zeros((NC, EMB, NBLK * P), np.float16)
    for c in range(NC):
        embT_pad[c, :, :NSH] = node_emb[c * NSH:(c + 1) * NSH].T.astype(np.float16)

    in_maps = []
    for c in range(NC):
        m = {
            "embT": embT_pad[c],
            "w1ext": w1ext,
            "w2ext": w2ext,
            "idxw": idx_w[c],
            "dlT": dl_T[c],
            "iota": iota,
            "iotar": iotar,
        }
        if b1_any:
            m["b1e"] = np.tile(b1[None, :], (P, 1)).astype(np.float32)
        if b2_any:
            m["b2e"] = np.tile(b2[None, :], (P, 1)).astype(np.float32)
        in_maps.append(m)

    res = run_bass_kernel_spmd(nc, in_maps, core_ids=list(range(NC)))
    out = np.concatenate([res.results[c]["out"] for c in range(NC)], axis=0)
    return np.ascontiguousarray(out.astype(np.float32))


if __name__ == "__main__":
    # quick self-exercise with random inputs of the right shapes
    rng = np.random.default_rng(0)
    ins = {
        "node_emb": rng.standard_normal((N, EMB), dtype=np.float32) * 0.05,
        "W1": rng.standard_normal((EMB, HIDDEN), dtype=np.float32) * 0.07,
        "att1_src": rng.standard_normal((HEADS, OUT1), dtype=np.float32) * 0.2,
        "att1_dst": rng.standard_normal((HEADS, OUT1), dtype=np.float32) * 0.2,
        "b1": np.zeros(HIDDEN, np.float32),
        "W2": rng.standard_normal((HIDDEN, REPR), dtype=np.float32) * 0.07,
        "att2_src": rng.standard_normal((1, REPR), dtype=np.float32) * 0.2,
        "att2_dst": rng.standard_normal((1, REPR), dtype=np.float32) * 0.2,
        "b2": np.zeros(REPR, np.float32),
        "edge_index": rng.integers(0, N, (2, E)).astype(np.int32),
    }
    out = kernel(**ins)
    print("out", out.shape, out.dtype, np.abs(out).mean())


# revision 16
# speedup vs baseline: 1.2599x; 1.2599x over previous
"""GAT (2-layer) on 8 Trainium2 NeuronCores — edge-parallel by destination.

Strategy
--------
- Nodes are sharded 8 ways (6250/core). Edges (incl. self-loops) are routed to
  the core that owns their destination node, so each core's scatter-adds are
  purely local (no all-reduce of aggregates).
- Phase A (per core): GEMM over its node shard computes h1 plus the attention
  logit projections a_src/a_dst (folded into the GEMM as extra output columns);
  rows [h1|a_src] are packed fp16 into a 768B-stride table, AllGathered so every
  core holds the full table.
- Edge phase (per core): edges are grouped by (128-node dst block, src-half) and
  chunked 128 at a time. Per chunk: dma_gather of the 128 source rows; a one-hot
  (edge x node) matrix built by is_equal against an iota constant; a_dst
  expanded edge-wise via a small matmul with the transposed one-hot; logits ->
  LeakyReLU(0.2) -> exp on the scalar engine; messages = h1_src * exp; the
  weighted scatter-sum is one fp16 matmul accumulating [128 nodes x (msg|exp)]
  in PSUM over the block (softmax denominator rides along as an extra column).
  Normalization (divide by the exp-sum) happens once per node at the end
  (softmax is shift-invariant; logits are O(1) so no max-subtraction needed).
- Between layers: ELU, second GEMM (x @ W2ext), second table, AllGather, same
  edge phase with 1 head, then the final normalize produces the output shard.
"""

import math
import os

import numpy as np

import concourse.bass as bass
import concourse.mybir as mybir
import concourse.tile as tile
from concourse import bacc
from concourse.bass_utils import run_bass_kernel_spmd
from concourse.masks import make_identity

# problem constants (from the reference)
N = 50000
E = 500000
EMB = 128
HIDDEN = 256
HEADS = 8
OUT1 = 32
REPR = 64
NEG_SLOPE = 0.2

NC = 8
P = 128
NSH = N // NC                    # 6250 nodes per core
NBLK = (NSH + P - 1) // P        # 49 dst blocks per core
LASTB = NSH - (NBLK - 1) * P     # 106 nodes in last block
HALF = N // 2                    # src table split (int16 gather indices)

T1_ELEM = 384                    # fp16: h1(256) | a_src(8) | pad -> 768B rows
T2_ELEM = 256                    # fp16: h2(64) | a2_src(1) | pad -> 512B rows

F16 = mybir.dt.float16
F32 = mybir.dt.float32
I16 = mybir.dt.int16
AF = mybir.ActivationFunctionType
ALU = mybir.AluOpType

MAX_CH_PER_CALL = 7              # 896 idxs/call, under the 1024-desc SWDGE ring


def _prep_edges(edge_index):
    """Partition + sort edges; build per-core gather-index / dst-local arrays."""
    ei = np.asarray(edge_index)
    src = np.concatenate([ei[0], np.arange(N, dtype=np.int64)]).astype(np.int64)
    dst = np.concatenate([ei[1], np.arange(N, dtype=np.int64)]).astype(np.int64)

    core = dst // NSH
    per_core = []
    kch = 1
    for c in range(NC):
        m = core == c
        s, d = src[m], dst[m] - c * NSH
        blk = d >> 7
        half = (s >= HALF).astype(np.int64)
        key = blk * 2 + half
        order = np.argsort(key, kind="stable")
        s, d, key = s[order], d[order], key[order]
        counts = np.bincount(key, minlength=NBLK * 2)
        kch = max(kch, int(math.ceil(counts.max() / P)))
        per_core.append((s, d, key, counts))

    nchunk = NBLK * 2 * kch
    idx_all = np.zeros((NC, NBLK * 2, kch * P), np.int16)
    dl_all = np.full((NC, NBLK * 2, kch * P), 200.0, np.float16)
    for c in range(NC):
        s, d, key, counts = per_core[c]
        starts = np.zeros(NBLK * 2 + 1, np.int64)
        np.cumsum(counts, out=starts[1:])
        for g in range(NBLK * 2):
            n = counts[g]
            if n == 0:
                continue
            sl = slice(starts[g], starts[g] + n)
            h = g & 1
            idx_all[c, g, :n] = (s[sl] - h * HALF).astype(np.int16)
            dl_all[c, g, :n] = (d[sl] & 127).astype(np.float16)

    # wrap gather indices per (block-pair, half) call: the call covers the
    # half's chunks of two consecutive blocks (7+7); idx i -> [i%16, i//16],
    # replicated 8x to 128 rows (one copy per Q7 core)
    npair = (NBLK + 1) // 2
    call_ch = [kch]              # kept for signature compat
    wcols = 2 * kch * P // 16    # idx columns per (pair, half) call
    idx_w = np.zeros((NC, 128, npair * 2 * wcols), np.int16)
    for c in range(NC):
        for bp in range(npair):
            for h in (0, 1):
                g0 = (2 * bp) * 2 + h
                parts = [idx_all[c, g0]]
                if 2 * bp + 1 < NBLK:
                    parts.append(idx_all[c, (2 * bp + 1) * 2 + h])
                else:
                    parts.append(np.zeros(kch * P, np.int16))
                col0 = (bp * 2 + h) * wcols
                for j, part in enumerate(parts):
                    w = part.reshape(len(part) // 16, 16).T
                    c0 = col0 + j * (kch * P // 16)
                    idx_w[c, :, c0:c0 + kch * P // 16] = np.tile(w, (8, 1))
    # dst-local per chunk, edge i of chunk -> partition i: [128, nchunk]
    dl_T = np.ascontiguousarray(
        dl_all.reshape(NC, NBLK * 2 * kch, P).transpose(0, 2, 1))
    return kch, call_ch, idx_w, dl_T


def _build(kch, call_ch, b1_any, b2_any):
    nchunk = NBLK * 2 * kch
    cpb = 2 * kch                # chunks per dst block
    npair = (NBLK + 1) // 2
    wcols = 2 * kch * P // 16    # idx cols per (pair, half) call
    nc = bacc.Bacc(None, target_bir_lowering=False)

    # ---- inputs (per core) ----
    embT_in = nc.dram_tensor("embT", [EMB, NBLK * P], F16, kind="ExternalInput")
    w1_in = nc.dram_tensor("w1ext", [EMB, HIDDEN + 16], F16, kind="ExternalInput")
    w2_in = nc.dram_tensor("w2ext", [HIDDEN, REPR + 2], F16, kind="ExternalInput")
    idx_in = nc.dram_tensor("idxw", [128, npair * 2 * wcols], I16, kind="ExternalInput")
    dl_in = nc.dram_tensor("dlT", [128, nchunk], F16, kind="ExternalInput")
    iota_in = nc.dram_tensor("iota", [P, P], F16, kind="ExternalInput")
    iotar_in = nc.dram_tensor("iotar", [P, cpb * P], F16, kind="ExternalInput")
    if b1_any:
        b1_in = nc.dram_tensor("b1e", [P, HIDDEN], F32, kind="ExternalInput")
    if b2_any:
        b2_in = nc.dram_tensor("b2e", [P, REPR], F32, kind="ExternalInput")
    out_t = nc.dram_tensor("out", [NSH, REPR], F32, kind="ExternalOutput")

    # ---- internal DRAM ----
    t1_shard = nc.dram_tensor("t1_shard", [NSH, T1_ELEM], F16, kind="Internal")
    t1_full = nc.dram_tensor("t1_full", [N, T1_ELEM], F16, kind="Internal",
                             addr_space="Shared")
    t2_shard = nc.dram_tensor("t2_shard", [NSH, T2_ELEM], F16, kind="Internal")
    t2_full = nc.dram_tensor("t2_full", [N, T2_ELEM], F16, kind="Internal",
                             addr_space="Shared")

    with tile.TileContext(nc) as tc:
        with (
            tc.tile_pool(name="const", bufs=1) as cst,
            tc.tile_pool(name="sb", bufs=3) as sb,
            tc.tile_pool(name="gp", bufs=6) as gp,
            tc.tile_pool(name="rp", bufs=3) as rp,
            tc.tile_pool(name="sb2", bufs=3) as sb2,
            tc.tile_pool(name="oh", bufs=3) as ohp,
            tc.tile_pool(name="psA", bufs=3, space="PSUM") as psA,
            tc.tile_pool(name="psB", bufs=2, space="PSUM") as psB,
            tc.tile_pool(name="psC", bufs=1, space="PSUM") as psC,
            tc.tile_pool(name="psD", bufs=2, space="PSUM") as psD,
        ):
            # ---- constants ----
            iota = cst.tile([P, P], F16)
            nc.sync.dma_start(out=iota[:], in_=iota_in[:])
            iotar = cst.tile([P, cpb * P], F16)
            nc.sync.dma_start(out=iotar[:], in_=iotar_in[:])
            ident = cst.tile([P, P], F16)
            make_identity(nc, ident[:])
            w1 = cst.tile([EMB, HIDDEN + 16], F16)
            nc.sync.dma_start(out=w1[:], in_=w1_in[:])
            w2 = cst.tile([P, 2, REPR + 2], F16)
            nc.sync.dma_start(out=w2[:, 0, :], in_=w2_in[0:P, :])
            nc.sync.dma_start(out=w2[:, 1, :], in_=w2_in[P:HIDDEN, :])
            it_all = cst.tile([128, npair * 2 * wcols], I16)
            nc.sync.dma_start(out=it_all[:], in_=idx_in[:])
            dl_all = cst.tile([128, nchunk], F16)
            nc.sync.dma_start(out=dl_all[:], in_=dl_in[:])
            adst1 = cst.tile([P, NBLK * 8], F16)
            adst2 = cst.tile([P, NBLK], F16)
            if b1_any:
                b1e = cst.tile([P, HIDDEN], F32)
                nc.sync.dma_start(out=b1e[:], in_=b1_in[:])
            if b2_any:
                b2e = cst.tile([P, REPR], F32)
                nc.sync.dma_start(out=b2e[:], in_=b2_in[:])

            # ---- phase A: h1 GEMM + table build (4 blocks per DMA) ----
            GA = 4
            for b0 in range(0, NBLK, GA):
                nb = min(GA, NBLK - b0)
                et = sb.tile([EMB, GA, P], F16, tag="embT")
                nc.sync.dma_start(
                    out=et[:, 0:nb, :].rearrange("p a n -> p (a n)"),
                    in_=embT_in[:, b0 * P:(b0 + nb) * P])
                t1s = sb.tile([P, GA, T1_ELEM], F16, tag="t1s")
                for j in range(nb):
                    b = b0 + j
                    ph1 = psA.tile([P, HIDDEN + 16], F32, tag="acc")
                    nc.tensor.matmul(out=ph1[:], lhsT=et[:, j, :], rhs=w1[:],
                                     start=True, stop=True)
                    nc.vector.tensor_copy(out=t1s[:, j, 0:HIDDEN],
                                          in_=ph1[:, 0:HIDDEN])
                    nc.scalar.copy(out=t1s[:, j, HIDDEN:HIDDEN + 8],
                                   in_=ph1[:, HIDDEN:HIDDEN + 8])
                    nc.vector.tensor_copy(out=adst1[:, b * 8:(b + 1) * 8],
                                          in_=ph1[:, HIDDEN + 8:HIDDEN + 16])
                full = nb if b0 + nb < NBLK else nb - 1
                if full:
                    nc.sync.dma_start(
                        out=t1_shard[b0 * P:(b0 + full) * P, :]
                            .rearrange("(a p) e -> p a e", p=P),
                        in_=t1s[:, 0:full, :])
                if full < nb:
                    nc.sync.dma_start(
                        out=t1_shard[(b0 + full) * P:(b0 + full) * P + LASTB, :],
                        in_=t1s[:LASTB, full, :])

            nc.gpsimd.collective_compute(
                "AllGather", ALU.bypass, ins=[t1_shard[:]], outs=[t1_full[:]],
                replica_groups=[list(range(NC))])

            # ---- edge phase helper ----
            def edge_layer(t_full, elem, hid, heads, adst_t, out_cb):
                """One GAT message-passing layer over this core's dst blocks.

                t_full: gather table [N, elem] fp16, row = [feat(hid)|a_src(heads)|pad]
                adst_t: [P, NBLK*heads] per-block a_dst values
                out_cb(b, ps_acc): consume the accumulated [P, hid+heads] psum
                """
                mcols = hid + heads          # matmul rhs columns (msg | exp)
                hw_half = kch * P // 16      # idx cols per half within a call
                PRE = 2                      # gather prefetch distance (blocks)
                gq = {}

                def issue_gathers(b):
                    bp, j = b // 2, b % 2
                    gs = []
                    for h in (0, 1):
                        gt = gp.tile([P, kch, elem], F16, tag=f"g{hid}")
                        col0 = (bp * 2 + h) * wcols + j * hw_half
                        nc.gpsimd.dma_gather(
                            out_ap=gt[:],
                            in_ap=t_full[h * HALF:(h + 1) * HALF, :],
                            idxs_ap=it_all[:, col0:col0 + hw_half],
                            num_idxs=kch * P, num_idxs_reg=kch * P,
                            elem_size=elem)
                        gs.append((gt[:], kch))
                    gq[b] = gs

                for b in range(min(PRE, NBLK)):
                    issue_gathers(b)
                for b in range(NBLK):
                    if b + PRE < NBLK:
                        issue_gathers(b + PRE)
                    gs = gq.pop(b)
                    # one-hots (one batched is_equal) + transposed one-hots
                    pse = psC.tile([P, cpb * heads], F32, tag="adst")
                    oh_all = ohp.tile([P, cpb, P], F16, tag="oh")
                    nc.vector.tensor_tensor(
                        out=oh_all[:],
                        in0=dl_all[:, b * cpb:(b + 1) * cpb]
                            .rearrange("p (t o) -> p t o", o=1)
                            .to_broadcast([P, cpb, P]),
                        in1=iotar[:].rearrange("p (t n) -> p t n", n=P),
                        op=ALU.is_equal)
                    ohs = [oh_all[:, k, :] for k in range(cpb)]
                    ohT_sb = ohp.tile([P, cpb, P], F16, tag="ohT_sb")
                    PSB_CH = 8   # chunks per fp16 psum bank
                    for g0 in range(0, cpb, PSB_CH):
                        g1 = min(g0 + PSB_CH, cpb)
                        pst = psB.tile([P, PSB_CH, P], F16, tag="ohT")
                        for k in range(g0, g1):
                            nc.tensor.transpose(out=pst[:, k - g0, :], in_=ohs[k],
                                                identity=ident[:])
                        nc.scalar.copy(out=ohT_sb[:, g0:g1, :].rearrange("p t n -> p (t n)"),
                                       in_=pst[:, 0:g1 - g0, :].rearrange("p t n -> p (t n)"))
                    for k in range(cpb):
                        nc.tensor.matmul(
                            out=pse[:, k * heads:(k + 1) * heads],
                            lhsT=ohT_sb[:, k, :],
                            rhs=adst_t[:, b * heads:(b + 1) * heads],
                            start=True, stop=True)
                    # logits -> leaky -> exp  (batched over the block's chunks)
                    e_sb = sb2.tile([P, cpb * heads], F32, tag=f"e{hid}")
                    k = 0
                    for gt, t in gs:
                        nc.vector.tensor_tensor(
                            out=e_sb[:, k * heads:(k + t) * heads]
                                .rearrange("p (t h) -> p t h", t=t),
                            in0=gt[:, :, hid:hid + heads],
                            in1=pse[:, k * heads:(k + t) * heads]
                                .rearrange("p (t h) -> p t h", t=t),
                            op=ALU.add)
                        k += t
                    lk = sb2.tile([P, cpb * heads], F32, tag=f"lk{hid}")
                    nc.scalar.activation(out=lk[:], in_=e_sb[:], func=AF.Prelu,
                                         alpha=NEG_SLOPE)
                    ex = sb2.tile([P, cpb * heads], F16, tag=f"ex{hid}")
                    nc.scalar.activation(out=ex[:], in_=lk[:], func=AF.Exp)
                    # messages (feat * exp, broadcast over feat/head) + exp col
                    rhs = rp.tile([P, cpb, mcols], F16, tag=f"rhs{hid}")
                    k = 0
                    for gi, (gt, t) in enumerate(gs):
                        # balance: route one L1 half's multiply to GPSIMD
                        eng = nc.gpsimd if (hid == HIDDEN and gi == 1) else nc.vector
                        eng.tensor_tensor(
                            out=rhs[:, k:k + t, 0:hid]
                                .rearrange("p t (h d) -> p t h d", h=heads),
                            in0=gt[:, :, 0:hid]
                                .rearrange("p t (h d) -> p t h d", h=heads),
                            in1=ex[:, k * heads:(k + t) * heads]
                                .rearrange("p (t h) -> p t h", t=t)[:, :, :, None]
                                .to_broadcast([P, t, heads, hid // heads]),
                            op=ALU.mult)
                        k += t
                    nc.vector.tensor_copy(
                        out=rhs[:, :, hid:hid + heads],
                        in_=ex[:].rearrange("p (t h) -> p t h", t=cpb))
                    # scatter-accumulate into the block's psum
                    pacc = psA.tile([P, mcols], F32, tag="acc")
                    for k in range(cpb):
                        nc.tensor.matmul(out=pacc[:], lhsT=ohs[k],
                                         rhs=rhs[:, k, :], start=(k == 0),
                                         stop=(k == cpb - 1))
                    out_cb(b, pacc)

            # ---- layer 1 block finisher: normalize, ELU, GEMM2, T2 rows ----
            def finish1(b, pacc):
                rows = P if b < NBLK - 1 else LASTB
                se = sb.tile([P, HEADS], F32, tag="se")
                nc.vector.tensor_scalar_add(out=se[:], in0=pacc[:, HIDDEN:HIDDEN + 8],
                                            scalar1=1e-16)
                rec = sb.tile([P, HEADS], F32, tag="rec")
                nc.vector.reciprocal(out=rec[:], in_=se[:])
                v = sb.tile([P, HIDDEN], F32, tag="v")
                nc.vector.tensor_tensor(
                    out=v[:].rearrange("p (h d) -> p h d", h=HEADS),
                    in0=pacc[:, 0:HIDDEN].rearrange("p (h d) -> p h d", h=HEADS),
                    in1=rec[:, :, None].to_broadcast([P, HEADS, OUT1]),
                    op=ALU.mult)
                if b1_any:
                    nc.vector.tensor_tensor(out=v[:], in0=v[:], in1=b1e[:], op=ALU.add)
                # elu(v) = relu(v) + exp(min(v,0)) - 1
                r = sb.tile([P, HIDDEN], F32, tag="relu")
                nc.scalar.activation(out=r[:], in_=v[:], func=AF.Relu)
                mn = sb.tile([P, HIDDEN], F32, tag="mn")
                nc.vector.tensor_scalar_min(out=mn[:], in0=v[:], scalar1=0.0)
                em = sb.tile([P, HIDDEN], F32, tag="em")
                nc.scalar.activation(out=em[:], in_=mn[:], func=AF.Exp)
                x = sb.tile([P, HIDDEN], F32, tag="x")
                nc.vector.tensor_tensor(out=x[:], in0=r[:], in1=em[:], op=ALU.add)
                x16 = sb.tile([P, HIDDEN], F16, tag="x16")
                nc.vector.tensor_scalar_add(out=x16[:], in0=x[:], scalar1=-1.0)
                # GEMM2: h2 = x @ W2ext  (transpose x tiles for lhsT)
                xT = sb.tile([P, 2, P], F16, tag="xT")
                for k in range(2):
                    pst = psD.tile([P, P], F16, tag="misc")
                    nc.tensor.transpose(out=pst[:], in_=x16[:, k * P:(k + 1) * P],
                                        identity=ident[:])
                    nc.scalar.copy(out=xT[:, k, :], in_=pst[:])
                ph2 = psD.tile([P, REPR + 2], F32, tag="misc")
                for k in range(2):
                    nc.tensor.matmul(out=ph2[:], lhsT=xT[:, k, :], rhs=w2[:, k, :],
                                     start=(k == 0), stop=(k == 1))
                t2s = sb.tile([P, T2_ELEM], F16, tag="t2s")
                nc.scalar.copy(out=t2s[:, 0:REPR], in_=ph2[:, 0:REPR])
                nc.vector.tensor_copy(out=t2s[:, REPR:REPR + 1],
                                      in_=ph2[:, REPR:REPR + 1])
                nc.vector.tensor_copy(out=adst2[:, b:b + 1],
                                      in_=ph2[:, REPR + 1:REPR + 2])
                nc.sync.dma_start(out=t2_shard[b * P:b * P + rows, :],
                                  in_=t2s[:rows, :])

            edge_layer(t1_full, T1_ELEM, HIDDEN, HEADS, adst1, finish1)

            nc.gpsimd.collective_compute(
                "AllGather", ALU.bypass, ins=[t2_shard[:]], outs=[t2_full[:]],
                replica_groups=[list(range(NC))])

            # ---- layer 2 block finisher: normalize -> output ----
            def finish2(b, pacc):
                rows = P if b < NBLK - 1 else LASTB
                se = sb.tile([P, 1], F32, tag="se2")
                nc.vector.tensor_scalar_add(out=se[:], in0=pacc[:, REPR:REPR + 1],
                                            scalar1=1e-16)
                rec = sb.tile([P, 1], F32, tag="rec2")
                nc.vector.reciprocal(out=rec[:], in_=se[:])
                o = sb.tile([P, REPR], F32, tag="o")
                nc.scalar.activation(out=o[:], in_=pacc[:, 0:REPR], func=AF.Copy,
                                     scale=rec[:, 0:1])
                if b2_any:
                    nc.vector.tensor_tensor(out=o[:], in0=o[:], in1=b2e[:], op=ALU.add)
                nc.sync.dma_start(out=out_t[b * P:b * P + rows, :], in_=o[:rows, :])

            edge_layer(t2_full, T2_ELEM, REPR, 1, adst2, finish2)

    nc.finalize()
    globals()["LAST_NC"] = nc
    return nc


def kernel(**inputs):
    node_emb = np.asarray(inputs["node_emb"], np.float32)
    W1 = np.asarray(inputs["W1"], np.float32)
    att1_src = np.asarray(inputs["att1_src"], np.float32)
    att1_dst = np.asarray(inputs["att1_dst"], np.float32)
    b1 = np.asarray(inputs["b1"], np.float32)
    W2 = np.asarray(inputs["W2"], np.float32)
    att2_src = np.asarray(inputs["att2_src"], np.float32)
    att2_dst = np.asarray(inputs["att2_dst"], np.float32)
    b2 = np.asarray(inputs["b2"], np.float32)
    edge_index = np.asarray(inputs["edge_index"])

    kch, call_ch, idx_w, dl_T = _prep_edges(edge_index)

    # fold attention projections into the GEMMs: a_src = emb @ (W1 . att)
    A1s = np.einsum("ehd,hd->eh", W1.reshape(EMB, HEADS, OUT1), att1_src)
    A1d = np.einsum("ehd,hd->eh", W1.reshape(EMB, HEADS, OUT1), att1_dst)
    w1ext = np.concatenate([W1, A1s, A1d], axis=1).astype(np.float16)
    A2s = W2 @ att2_src[0]
    A2d = W2 @ att2_dst[0]
    w2ext = np.concatenate([W2, A2s[:, None], A2d[:, None]], axis=1).astype(np.float16)

    iota = np.tile(np.arange(P, dtype=np.float16), (P, 1))
    iotar = np.tile(np.arange(P, dtype=np.float16), (P, 2 * kch))
    b1_any = bool(np.any(b1))
    b2_any = bool(np.any(b2))

    nc = _build(kch, call_ch, b1_any, b2_any)

    embT_pad = np.zeros((NC, EMB, NBLK * P), np.float16)
    for c in range(NC):
        embT_pad[c, :, :NSH] = node_emb[c * NSH:(c + 1) * NSH].T.astype(np.float16)

    in_maps = []
    for c in range(NC):
        m = {
            "embT": embT_pad[c],
            "w1ext": w1ext,
            "w2ext": w2ext,
            "idxw": idx_w[c],
            "dlT": dl_T[c],
            "iota": iota,
            "iotar": iotar,
        }
        if b1_any:
            m["b1e"] = np.tile(b1[None, :], (P, 1)).astype(np.float32)
        if b2_any:
            m["b2e"] = np.tile(b2[None, :], (P, 1)).astype(np.float32)
        in_maps.append(m)

    res = run_bass_kernel_spmd(nc, in_maps, core_ids=list(range(NC)))
    out = np.concatenate([res.results[c]["out"] for c in range(NC)], axis=0)
    return np.ascontiguousarray(out.astype(np.float32))


if __name__ == "__main__":
    # quick self-exercise with random inputs of the right shapes
    rng = np.random.default_rng(0)
    ins = {
        "node_emb": rng.standard_normal((N, EMB), dtype=np.float32) * 0.05,
        "W1": rng.standard_normal((EMB, HIDDEN), dtype=np.float32) * 0.07,
        "att1_src": rng.standard_normal((HEADS, OUT1), dtype=np.float32) * 0.2,
        "att1_dst": rng.standard_normal((HEADS, OUT1), dtype=np.float32) * 0.2,
        "b1": np.zeros(HIDDEN, np.float32),
        "W2": rng.standard_normal((HIDDEN, REPR), dtype=np.float32) * 0.07,
        "att2_src": rng.standard_normal((1, REPR), dtype=np.float32) * 0.2,
        "att2_dst": rng.standard_normal((1, REPR), dtype=np.float32) * 0.2,
        "b2": np.zeros(REPR, np.float32),
        "edge_index": rng.integers(0, N, (2, E)).astype(np.int32),
    }
    out = kernel(**ins)
    print("out", out.shape, out.dtype, np.abs(out).mean())


# revision 20
# speedup vs baseline: 1.2964x; 1.0290x over previous
"""GAT (2-layer) on 8 Trainium2 NeuronCores — edge-parallel by destination.

Strategy
--------
- Nodes are sharded 8 ways (6250/core). Edges (incl. self-loops) are routed to
  the core that owns their destination node, so each core's scatter-adds are
  purely local (no all-reduce of aggregates).
- Phase A (per core): GEMM over its node shard computes h1 plus the attention
  logit projections a_src/a_dst (folded into the GEMM as extra output columns);
  rows [h1|a_src] are packed fp16 into a 768B-stride table, AllGathered so every
  core holds the full table.
- Edge phase (per core): edges are grouped by (128-node dst block, src-half) and
  chunked 128 at a time. Per chunk: dma_gather of the 128 source rows; a one-hot
  (edge x node) matrix built by is_equal against an iota constant; a_dst
  expanded edge-wise via a small matmul with the transposed one-hot; logits ->
  LeakyReLU(0.2) -> exp on the scalar engine; messages = h1_src * exp; the
  weighted scatter-sum is one fp16 matmul accumulating [128 nodes x (msg|exp)]
  in PSUM over the block (softmax denominator rides along as an extra column).
  Normalization (divide by the exp-sum) happens once per node at the end
  (softmax is shift-invariant; logits are O(1) so no max-subtraction needed).
- Between layers: ELU, second GEMM (x @ W2ext), second table, AllGather, same
  edge phase with 1 head, then the final normalize produces the output shard.
"""

import math
import os

import numpy as np

import concourse.bass as bass
import concourse.mybir as mybir
import concourse.tile as tile
from concourse import bacc
from concourse.bass_utils import run_bass_kernel_spmd
from concourse.masks import make_identity

# problem constants (from the reference)
N = 50000
E = 500000
EMB = 128
HIDDEN = 256
HEADS = 8
OUT1 = 32
REPR = 64
NEG_SLOPE = 0.2

NC = 8
P = 128
NSH = N // NC                    # 6250 nodes per core
NBLK = (NSH + P - 1) // P        # 49 dst blocks per core
LASTB = NSH - (NBLK - 1) * P     # 106 nodes in last block
HALF_B_CANDIDATES = range(25000, 28251, 250)   # src table split candidates
                                               # (both sides < 32768 rows)

T1_ELEM = 384                    # fp16: h1(256) | a_src(8) | pad -> 768B rows
T2_ELEM = 256                    # fp16: h2(64) | a2_src(1) | pad -> 512B rows

F16 = mybir.dt.float16
F32 = mybir.dt.float32
I16 = mybir.dt.int16
AF = mybir.ActivationFunctionType
ALU = mybir.AluOpType

MAX_CH_PER_CALL = 7              # 896 idxs/call, under the 1024-desc SWDGE ring


def _prep_edges(edge_index):
    """Partition + sort edges; build per-core gather-index / dst-local arrays."""
    ei = np.asarray(edge_index)
    src = np.concatenate([ei[0], np.arange(N, dtype=np.int64)]).astype(np.int64)
    dst = np.concatenate([ei[1], np.arange(N, dtype=np.int64)]).astype(np.int64)

    core = dst // NSH
    # pick the src-half boundary minimizing total chunks per block (padding)
    best = None
    for B in HALF_B_CANDIDATES:
        m0 = m1 = 1
        for c in range(NC):
            m = core == c
            s, d = src[m], dst[m] - c * NSH
            key = (d >> 7) * 2 + (s >= B)
            counts = np.bincount(key, minlength=NBLK * 2)
            m0 = max(m0, int(counts[0::2].max()))
            m1 = max(m1, int(counts[1::2].max()))
        k0, k1 = math.ceil(m0 / P), math.ceil(m1 / P)
        margin = min(k0 * P - m0, k1 * P - m1)
        cand = (k0 + k1, -margin, B, k0, k1)
        if best is None or cand < best:
            best = cand
    _, _, half_b, kch0, kch1 = best

    per_core = []
    for c in range(NC):
        m = core == c
        s, d = src[m], dst[m] - c * NSH
        key = (d >> 7) * 2 + (s >= half_b)
        order = np.argsort(key, kind="stable")
        s, d, key = s[order], d[order], key[order]
        counts = np.bincount(key, minlength=NBLK * 2)
        per_core.append((s, d, key, counts))
    kchs = (kch0, kch1)
    kmax = max(kch0, kch1)
    idx_all = np.zeros((NC, NBLK * 2, kmax * P), np.int16)
    dl_all = np.full((NC, NBLK * 2, kmax * P), 200.0, np.float16)
    for c in range(NC):
        s, d, key, counts = per_core[c]
        starts = np.zeros(NBLK * 2 + 1, np.int64)
        np.cumsum(counts, out=starts[1:])
        for g in range(NBLK * 2):
            n = counts[g]
            if n == 0:
                continue
            sl = slice(starts[g], starts[g] + n)
            h = g & 1
            idx_all[c, g, :n] = (s[sl] - h * half_b).astype(np.int16)
            dl_all[c, g, :n] = (d[sl] & 127).astype(np.float16)

    # wrap gather indices per (block, half) call: idx i -> [i%16, i//16],
    # replicated 8x to 128 rows (one copy per Q7 core)
    w0, w1 = kch0 * P // 16, kch1 * P // 16
    idx_w = np.zeros((NC, 128, NBLK * (w0 + w1)), np.int16)
    cpb = kch0 + kch1
    dl_T = np.full((NC, 128, NBLK * cpb), 200.0, np.float16)
    for c in range(NC):
        for b in range(NBLK):
            for h, (kch, woff) in enumerate(((kch0, 0), (kch1, w0))):
                part = idx_all[c, b * 2 + h, :kch * P]
                w = part.reshape(kch * P // 16, 16).T
                c0 = b * (w0 + w1) + woff
                idx_w[c, :, c0:c0 + kch * P // 16] = np.tile(w, (8, 1))
                dpart = dl_all[c, b * 2 + h, :kch * P].reshape(kch, P)
                ci0 = b * cpb + (0 if h == 0 else kch0)
                dl_T[c, :, ci0:ci0 + kch] = dpart.T
    return kchs, half_b, idx_w, dl_T


def _build(kchs, half_b, b1_any, b2_any):
    kch0, kch1 = kchs
    cpb = kch0 + kch1            # chunks per dst block
    nchunk = NBLK * cpb
    iw0, iw1 = kch0 * P // 16, kch1 * P // 16
    nc = bacc.Bacc(None, target_bir_lowering=False)

    # ---- inputs (per core) ----
    embT_in = nc.dram_tensor("embT", [EMB, NBLK * P], F16, kind="ExternalInput")
    w1_in = nc.dram_tensor("w1ext", [EMB, HIDDEN + 16], F16, kind="ExternalInput")
    w2_in = nc.dram_tensor("w2ext", [HIDDEN, REPR + 2], F16, kind="ExternalInput")
    idx_in = nc.dram_tensor("idxw", [128, NBLK * (iw0 + iw1)], I16, kind="ExternalInput")
    dl_in = nc.dram_tensor("dlT", [128, nchunk], F16, kind="ExternalInput")
    iota_in = nc.dram_tensor("iota", [P, P], F16, kind="ExternalInput")
    iotar_in = nc.dram_tensor("iotar", [P, cpb * P], F16, kind="ExternalInput")
    if b1_any:
        b1_in = nc.dram_tensor("b1e", [P, HIDDEN], F32, kind="ExternalInput")
    if b2_any:
        b2_in = nc.dram_tensor("b2e", [P, REPR], F32, kind="ExternalInput")
    out_t = nc.dram_tensor("out", [NSH, REPR], F32, kind="ExternalOutput")

    # ---- internal DRAM ----
    t1_shard = nc.dram_tensor("t1_shard", [NSH, T1_ELEM], F16, kind="Internal")
    t1_full = nc.dram_tensor("t1_full", [N, T1_ELEM], F16, kind="Internal",
                             addr_space="Shared")
    t2_shard = nc.dram_tensor("t2_shard", [NSH, T2_ELEM], F16, kind="Internal")
    t2_full = nc.dram_tensor("t2_full", [N, T2_ELEM], F16, kind="Internal",
                             addr_space="Shared")

    with tile.TileContext(nc) as tc:
        with (
            tc.tile_pool(name="const", bufs=1) as cst,
            tc.tile_pool(name="sb", bufs=3) as sb,
            tc.tile_pool(name="gp", bufs=6) as gp,
            tc.tile_pool(name="rp", bufs=3) as rp,
            tc.tile_pool(name="sb2", bufs=3) as sb2,
            tc.tile_pool(name="oh", bufs=3) as ohp,
            tc.tile_pool(name="psA", bufs=3, space="PSUM") as psA,
            tc.tile_pool(name="psB", bufs=2, space="PSUM") as psB,
            tc.tile_pool(name="psC", bufs=1, space="PSUM") as psC,
            tc.tile_pool(name="psD", bufs=2, space="PSUM") as psD,
        ):
            # ---- constants ----
            iota = cst.tile([P, P], F16)
            nc.sync.dma_start(out=iota[:], in_=iota_in[:])
            iotar = cst.tile([P, cpb * P], F16)
            nc.sync.dma_start(out=iotar[:], in_=iotar_in[:])
            ident = cst.tile([P, P], F16)
            make_identity(nc, ident[:])
            w1 = cst.tile([EMB, HIDDEN + 16], F16)
            nc.sync.dma_start(out=w1[:], in_=w1_in[:])
            w2 = cst.tile([P, 2, REPR + 2], F16)
            nc.sync.dma_start(out=w2[:, 0, :], in_=w2_in[0:P, :])
            nc.sync.dma_start(out=w2[:, 1, :], in_=w2_in[P:HIDDEN, :])
            it_all = cst.tile([128, NBLK * (iw0 + iw1)], I16)
            nc.sync.dma_start(out=it_all[:], in_=idx_in[:])
            dl_all = cst.tile([128, nchunk], F16)
            nc.sync.dma_start(out=dl_all[:], in_=dl_in[:])
            adst1 = cst.tile([P, NBLK * 8], F16)
            adst2 = cst.tile([P, NBLK], F16)
            if b1_any:
                b1e = cst.tile([P, HIDDEN], F32)
                nc.sync.dma_start(out=b1e[:], in_=b1_in[:])
            if b2_any:
                b2e = cst.tile([P, REPR], F32)
                nc.sync.dma_start(out=b2e[:], in_=b2_in[:])

            # ---- phase A: h1 GEMM + table build (4 blocks per DMA) ----
            GA = 4
            for b0 in range(0, NBLK, GA):
                nb = min(GA, NBLK - b0)
                et = sb.tile([EMB, GA, P], F16, tag="embT")
                nc.sync.dma_start(
                    out=et[:, 0:nb, :].rearrange("p a n -> p (a n)"),
                    in_=embT_in[:, b0 * P:(b0 + nb) * P])
                t1s = sb.tile([P, GA, T1_ELEM], F16, tag="t1s")
                for j in range(nb):
                    b = b0 + j
                    ph1 = psA.tile([P, HIDDEN + 16], F32, tag="acc")
                    nc.tensor.matmul(out=ph1[:], lhsT=et[:, j, :], rhs=w1[:],
                                     start=True, stop=True)
                    nc.vector.tensor_copy(out=t1s[:, j, 0:HIDDEN],
                                          in_=ph1[:, 0:HIDDEN])
                    nc.scalar.copy(out=t1s[:, j, HIDDEN:HIDDEN + 8],
                                   in_=ph1[:, HIDDEN:HIDDEN + 8])
                    nc.vector.tensor_copy(out=adst1[:, b * 8:(b + 1) * 8],
                                          in_=ph1[:, HIDDEN + 8:HIDDEN + 16])
                full = nb if b0 + nb < NBLK else nb - 1
                if full:
                    nc.sync.dma_start(
                        out=t1_shard[b0 * P:(b0 + full) * P, :]
                            .rearrange("(a p) e -> p a e", p=P),
                        in_=t1s[:, 0:full, :])
                if full < nb:
                    nc.sync.dma_start(
                        out=t1_shard[(b0 + full) * P:(b0 + full) * P + LASTB, :],
                        in_=t1s[:LASTB, full, :])

            nc.gpsimd.collective_compute(
                "AllGather", ALU.bypass, ins=[t1_shard[:]], outs=[t1_full[:]],
                replica_groups=[list(range(NC))])

            # ---- edge phase helper ----
            def edge_layer(t_full, elem, hid, heads, adst_t, out_cb):
                """One GAT message-passing layer over this core's dst blocks.

                t_full: gather table [N, elem] fp16, row = [feat(hid)|a_src(heads)|pad]
                adst_t: [P, NBLK*heads] per-block a_dst values
                out_cb(b, ps_acc): consume the accumulated [P, hid+heads] psum
                """
                mcols = hid + heads          # matmul rhs columns (msg | exp)
                PRE = 2                      # gather prefetch distance (blocks)
                gq = {}

                def issue_gathers(b):
                    gs = []
                    for h, (kch, woff, r0, r1) in enumerate((
                            (kch0, 0, 0, half_b), (kch1, iw0, half_b, N))):
                        gt = gp.tile([P, kch, elem], F16, tag=f"g{hid}{h}")
                        col0 = b * (iw0 + iw1) + woff
                        # <=7 chunks (896 idxs) per call: the SWDGE descriptor
                        # ring holds 1024
                        for off in range(0, kch, 7):
                            t = min(7, kch - off)
                            nc.gpsimd.dma_gather(
                                out_ap=gt[:, off:off + t, :],
                                in_ap=t_full[r0:r1, :],
                                idxs_ap=it_all[:, col0 + off * 8:
                                               col0 + (off + t) * 8],
                                num_idxs=t * P, num_idxs_reg=t * P,
                                elem_size=elem)
                        gs.append((gt[:], kch))
                    gq[b] = gs

                for b in range(min(PRE, NBLK)):
                    issue_gathers(b)
                for b in range(NBLK):
                    if b + PRE < NBLK:
                        issue_gathers(b + PRE)
                    gs = gq.pop(b)
                    # one-hots (one batched is_equal) + transposed one-hots
                    pse = psC.tile([P, cpb * heads], F32, tag="adst")
                    oh_all = ohp.tile([P, cpb, P], F16, tag="oh")
                    nc.vector.tensor_tensor(
                        out=oh_all[:],
                        in0=dl_all[:, b * cpb:(b + 1) * cpb]
                            .rearrange("p (t o) -> p t o", o=1)
                            .to_broadcast([P, cpb, P]),
                        in1=iotar[:].rearrange("p (t n) -> p t n", n=P),
                        op=ALU.is_equal)
                    ohs = [oh_all[:, k, :] for k in range(cpb)]
                    ohT_sb = ohp.tile([P, cpb, P], F16, tag="ohT_sb")
                    PSB_CH = 8   # chunks per fp16 psum bank
                    for g0 in range(0, cpb, PSB_CH):
                        g1 = min(g0 + PSB_CH, cpb)
                        pst = psB.tile([P, PSB_CH, P], F16, tag="ohT")
                        for k in range(g0, g1):
                            nc.tensor.transpose(out=pst[:, k - g0, :], in_=ohs[k],
                                                identity=ident[:])
                        nc.scalar.copy(out=ohT_sb[:, g0:g1, :].rearrange("p t n -> p (t n)"),
                                       in_=pst[:, 0:g1 - g0, :].rearrange("p t n -> p (t n)"))
                    for k in range(cpb):
                        nc.tensor.matmul(
                            out=pse[:, k * heads:(k + 1) * heads],
                            lhsT=ohT_sb[:, k, :],
                            rhs=adst_t[:, b * heads:(b + 1) * heads],
                            start=True, stop=True)
                    # logits -> leaky -> exp  (batched over the block's chunks)
                    e_sb = sb2.tile([P, cpb * heads], F32, tag=f"e{hid}")
                    k = 0
                    for gt, t in gs:
                        nc.vector.tensor_tensor(
                            out=e_sb[:, k * heads:(k + t) * heads]
                                .rearrange("p (t h) -> p t h", t=t),
                            in0=gt[:, :, hid:hid + heads],
                            in1=pse[:, k * heads:(k + t) * heads]
                                .rearrange("p (t h) -> p t h", t=t),
                            op=ALU.add)
                        k += t
                    lk = sb2.tile([P, cpb * heads], F32, tag=f"lk{hid}")
                    nc.scalar.activation(out=lk[:], in_=e_sb[:], func=AF.Prelu,
                                         alpha=NEG_SLOPE)
                    ex = sb2.tile([P, cpb * heads], F16, tag=f"ex{hid}")
                    nc.scalar.activation(out=ex[:], in_=lk[:], func=AF.Exp)
                    # messages (feat * exp, broadcast over feat/head) + exp col
                    rhs = rp.tile([P, cpb, mcols], F16, tag=f"rhs{hid}")
                    k = 0
                    for gi, (gt, t) in enumerate(gs):
                        # balance: route one L1 half's multiply to GPSIMD
                        eng = nc.gpsimd if (hid == HIDDEN and gi == 1) else nc.vector
                        eng.tensor_tensor(
                            out=rhs[:, k:k + t, 0:hid]
                                .rearrange("p t (h d) -> p t h d", h=heads),
                            in0=gt[:, :, 0:hid]
                                .rearrange("p t (h d) -> p t h d", h=heads),
                            in1=ex[:, k * heads:(k + t) * heads]
                                .rearrange("p (t h) -> p t h", t=t)[:, :, :, None]
                                .to_broadcast([P, t, heads, hid // heads]),
                            op=ALU.mult)
                        k += t
                    nc.vector.tensor_copy(
                        out=rhs[:, :, hid:hid + heads],
                        in_=ex[:].rearrange("p (t h) -> p t h", t=cpb))
                    # scatter-accumulate into the block's psum
                    pacc = psA.tile([P, mcols], F32, tag="acc")
                    for k in range(cpb):
                        nc.tensor.matmul(out=pacc[:], lhsT=ohs[k],
                                         rhs=rhs[:, k, :], start=(k == 0),
                                         stop=(k == cpb - 1))
                    out_cb(b, pacc)

            # ---- layer 1 block finisher: normalize, ELU, GEMM2, T2 rows ----
            def finish1(b, pacc):
                rows = P if b < NBLK - 1 else LASTB
                se = sb.tile([P, HEADS], F32, tag="se")
                nc.vector.tensor_scalar_add(out=se[:], in0=pacc[:, HIDDEN:HIDDEN + 8],
                                            scalar1=1e-16)
                rec = sb.tile([P, HEADS], F32, tag="rec")
                nc.vector.reciprocal(out=rec[:], in_=se[:])
                v = sb.tile([P, HIDDEN], F32, tag="v")
                nc.vector.tensor_tensor(
                    out=v[:].rearrange("p (h d) -> p h d", h=HEADS),
                    in0=pacc[:, 0:HIDDEN].rearrange("p (h d) -> p h d", h=HEADS),
                    in1=rec[:, :, None].to_broadcast([P, HEADS, OUT1]),
                    op=ALU.mult)
                if b1_any:
                    nc.vector.tensor_tensor(out=v[:], in0=v[:], in1=b1e[:], op=ALU.add)
                # elu(v) = relu(v) + exp(min(v,0)) - 1
                r = sb.tile([P, HIDDEN], F32, tag="relu")
                nc.scalar.activation(out=r[:], in_=v[:], func=AF.Relu)
                mn = sb.tile([P, HIDDEN], F32, tag="mn")
                nc.vector.tensor_scalar_min(out=mn[:], in0=v[:], scalar1=0.0)
                em = sb.tile([P, HIDDEN], F32, tag="em")
                nc.scalar.activation(out=em[:], in_=mn[:], func=AF.Exp)
                x = sb.tile([P, HIDDEN], F32, tag="x")
                nc.vector.tensor_tensor(out=x[:], in0=r[:], in1=em[:], op=ALU.add)
                x16 = sb.tile([P, HIDDEN], F16, tag="x16")
                nc.vector.tensor_scalar_add(out=x16[:], in0=x[:], scalar1=-1.0)
                # GEMM2: h2 = x @ W2ext  (transpose x tiles for lhsT)
                xT = sb.tile([P, 2, P], F16, tag="xT")
                for k in range(2):
                    pst = psD.tile([P, P], F16, tag="misc")
                    nc.tensor.transpose(out=pst[:], in_=x16[:, k * P:(k + 1) * P],
                                        identity=ident[:])
                    nc.scalar.copy(out=xT[:, k, :], in_=pst[:])
                ph2 = psD.tile([P, REPR + 2], F32, tag="misc")
                for k in range(2):
                    nc.tensor.matmul(out=ph2[:], lhsT=xT[:, k, :], rhs=w2[:, k, :],
                                     start=(k == 0), stop=(k == 1))
                t2s = sb.tile([P, T2_ELEM], F16, tag="t2s")
                nc.scalar.copy(out=t2s[:, 0:REPR], in_=ph2[:, 0:REPR])
                nc.vector.tensor_copy(out=t2s[:, REPR:REPR + 1],
                                      in_=ph2[:, REPR:REPR + 1])
                nc.vector.tensor_copy(out=adst2[:, b:b + 1],
                                      in_=ph2[:, REPR + 1:REPR + 2])
                nc.sync.dma_start(out=t2_shard[b * P:b * P + rows, :],
                                  in_=t2s[:rows, :])

            edge_layer(t1_full, T1_ELEM, HIDDEN, HEADS, adst1, finish1)

            nc.gpsimd.collective_compute(
                "AllGather", ALU.bypass, ins=[t2_shard[:]], outs=[t2_full[:]],
                replica_groups=[list(range(NC))])

            # ---- layer 2 block finisher: normalize -> output ----
            def finish2(b, pacc):
                rows = P if b < NBLK - 1 else LASTB
                se = sb.tile([P, 1], F32, tag="se2")
                nc.vector.tensor_scalar_add(out=se[:], in0=pacc[:, REPR:REPR + 1],
                                            scalar1=1e-16)
                rec = sb.tile([P, 1], F32, tag="rec2")
                nc.vector.reciprocal(out=rec[:], in_=se[:])
                o = sb.tile([P, REPR], F32, tag="o")
                nc.scalar.activation(out=o[:], in_=pacc[:, 0:REPR], func=AF.Copy,
                                     scale=rec[:, 0:1])
                if b2_any:
                    nc.vector.tensor_tensor(out=o[:], in0=o[:], in1=b2e[:], op=ALU.add)
                nc.sync.dma_start(out=out_t[b * P:b * P + rows, :], in_=o[:rows, :])

            edge_layer(t2_full, T2_ELEM, REPR, 1, adst2, finish2)

    nc.finalize()
    globals()["LAST_NC"] = nc
    return nc


def kernel(**inputs):
    node_emb = np.asarray(inputs["node_emb"], np.float32)
    W1 = np.asarray(inputs["W1"], np.float32)
    att1_src = np.asarray(inputs["att1_src"], np.float32)
    att1_dst = np.asarray(inputs["att1_dst"], np.float32)
    b1 = np.asarray(inputs["b1"], np.float32)
    W2 = np.asarray(inputs["W2"], np.float32)
    att2_src = np.asarray(inputs["att2_src"], np.float32)
    att2_dst = np.asarray(inputs["att2_dst"], np.float32)
    b2 = np.asarray(inputs["b2"], np.float32)
    edge_index = np.asarray(inputs["edge_index"])

    kchs, half_b, idx_w, dl_T = _prep_edges(edge_index)

    # fold attention projections into the GEMMs: a_src = emb @ (W1 . att)
    A1s = np.einsum("ehd,hd->eh", W1.reshape(EMB, HEADS, OUT1), att1_src)
    A1d = np.einsum("ehd,hd->eh", W1.reshape(EMB, HEADS, OUT1), att1_dst)
    w1ext = np.concatenate([W1, A1s, A1d], axis=1).astype(np.float16)
    A2s = W2 @ att2_src[0]
    A2d = W2 @ att2_dst[0]
    w2ext = np.concatenate([W2, A2s[:, None], A2d[:, None]], axis=1).astype(np.float16)

    iota = np.tile(np.arange(P, dtype=np.float16), (P, 1))
    iotar = np.tile(np.arange(P, dtype=np.float16), (P, sum(kchs)))
    b1_any = bool(np.any(b1))
    b2_any = bool(np.any(b2))

    nc = _build(kchs, half_b, b1_any, b2_any)

    embT_pad = np.zeros((NC, EMB, NBLK * P), np.float16)
    for c in range(NC):
        embT_pad[c, :, :NSH] = node_emb[c * NSH:(c + 1) * NSH].T.astype(np.float16)

    in_maps = []
    for c in range(NC):
        m = {
            "embT": embT_pad[c],
            "w1ext": w1ext,
            "w2ext": w2ext,
            "idxw": idx_w[c],
            "dlT": dl_T[c],
            "iota": iota,
            "iotar": iotar,
        }
        if b1_any:
            m["b1e"] = np.tile(b1[None, :], (P, 1)).astype(np.float32)
        if b2_any:
            m["b2e"] = np.tile(b2[None, :], (P, 1)).astype(np.float32)
        in_maps.append(m)

    res = run_bass_kernel_spmd(nc, in_maps, core_ids=list(range(NC)))
    out = np.concatenate([res.results[c]["out"] for c in range(NC)], axis=0)
    return np.ascontiguousarray(out.astype(np.float32))


if __name__ == "__main__":
    # quick self-exercise with random inputs of the right shapes
    rng = np.random.default_rng(0)
    ins = {
        "node_emb": rng.standard_normal((N, EMB), dtype=np.float32) * 0.05,
        "W1": rng.standard_normal((EMB, HIDDEN), dtype=np.float32) * 0.07,
        "att1_src": rng.standard_normal((HEADS, OUT1), dtype=np.float32) * 0.2,
        "att1_dst": rng.standard_normal((HEADS, OUT1), dtype=np.float32) * 0.2,
        "b1": np.zeros(HIDDEN, np.float32),
        "W2": rng.standard_normal((HIDDEN, REPR), dtype=np.float32) * 0.07,
        "att2_src": rng.standard_normal((1, REPR), dtype=np.float32) * 0.2,
        "att2_dst": rng.standard_normal((1, REPR), dtype=np.float32) * 0.2,
        "b2": np.zeros(REPR, np.float32),
        "edge_index": rng.integers(0, N, (2, E)).astype(np.int32),
    }
    out = kernel(**ins)
    print("out", out.shape, out.dtype, np.abs(out).mean())


# revision 26
# speedup vs baseline: 1.3218x; 1.0196x over previous
"""GAT (2-layer) on 8 Trainium2 NeuronCores — edge-parallel by destination.

Strategy
--------
- Nodes are sharded 8 ways (6250/core). Edges (incl. self-loops) are routed to
  the core that owns their destination node, so each core's scatter-adds are
  purely local (no all-reduce of aggregates).
- Phase A (per core): GEMM over its node shard computes h1 plus the attention
  logit projections a_src/a_dst (folded into the GEMM as extra output columns);
  rows [h1|a_src] are packed fp16 into a 768B-stride table, AllGathered so every
  core holds the full table.
- Edge phase (per core): edges are grouped by (128-node dst block, src-half) and
  chunked 128 at a time. Per chunk: dma_gather of the 128 source rows; a one-hot
  (edge x node) matrix built by is_equal against an iota constant; a_dst
  expanded edge-wise via a small matmul with the transposed one-hot; logits ->
  LeakyReLU(0.2) -> exp on the scalar engine; messages = h1_src * exp; the
  weighted scatter-sum is one fp16 matmul accumulating [128 nodes x (msg|exp)]
  in PSUM over the block (softmax denominator rides along as an extra column).
  Normalization (divide by the exp-sum) happens once per node at the end
  (softmax is shift-invariant; logits are O(1) so no max-subtraction needed).
- Between layers: ELU, second GEMM (x @ W2ext), second table, AllGather, same
  edge phase with 1 head, then the final normalize produces the output shard.
"""

import math
import os

import numpy as np

import concourse.bass as bass
import concourse.mybir as mybir
import concourse.tile as tile
from concourse import bacc
from concourse.bass_utils import run_bass_kernel_spmd
from concourse.masks import make_identity

# problem constants (from the reference)
N = 50000
E = 500000
EMB = 128
HIDDEN = 256
HEADS = 8
OUT1 = 32
REPR = 64
NEG_SLOPE = 0.2

NC = 8
P = 128
NSH = N // NC                    # 6250 nodes per core
NBLK = (NSH + P - 1) // P        # 49 dst blocks per core
LASTB = NSH - (NBLK - 1) * P     # 106 nodes in last block
HALF_B_CANDIDATES = range(25000, 28251, 250)   # src table split candidates
                                               # (both sides < 32768 rows)

T1_ELEM = 384                    # fp16: h1(256) | a_src(8) | pad -> 768B rows
T2_ELEM = 256                    # fp16: h2(64) | a2_src(1) | pad -> 512B rows

F16 = mybir.dt.float16
F32 = mybir.dt.float32
I16 = mybir.dt.int16
AF = mybir.ActivationFunctionType
ALU = mybir.AluOpType

MAX_CH_PER_CALL = 7              # 896 idxs/call, under the 1024-desc SWDGE ring


def _prep_edges(edge_index):
    """Partition + sort edges; build per-core gather-index / dst-local arrays."""
    ei = np.asarray(edge_index)
    src = np.concatenate([ei[0], np.arange(N, dtype=np.int64)]).astype(np.int64)
    dst = np.concatenate([ei[1], np.arange(N, dtype=np.int64)]).astype(np.int64)

    core = dst // NSH
    # pick the src-half boundary minimizing total chunks per block (padding)
    best = None
    for B in HALF_B_CANDIDATES:
        m0 = m1 = 1
        for c in range(NC):
            m = core == c
            s, d = src[m], dst[m] - c * NSH
            key = (d >> 7) * 2 + (s >= B)
            counts = np.bincount(key, minlength=NBLK * 2)
            m0 = max(m0, int(counts[0::2].max()))
            m1 = max(m1, int(counts[1::2].max()))
        k0, k1 = math.ceil(m0 / P), math.ceil(m1 / P)
        margin = min(k0 * P - m0, k1 * P - m1)
        cand = (k0 + k1, -margin, B, k0, k1)
        if best is None or cand < best:
            best = cand
    _, _, half_b, kch0, kch1 = best

    per_core = []
    for c in range(NC):
        m = core == c
        s, d = src[m], dst[m] - c * NSH
        key = (d >> 7) * 2 + (s >= half_b)
        order = np.argsort(key, kind="stable")
        s, d, key = s[order], d[order], key[order]
        counts = np.bincount(key, minlength=NBLK * 2)
        per_core.append((s, d, key, counts))
    kchs = (kch0, kch1)
    kmax = max(kch0, kch1)
    idx_all = np.zeros((NC, NBLK * 2, kmax * P), np.int16)
    dl_all = np.full((NC, NBLK * 2, kmax * P), 200.0, np.float16)
    for c in range(NC):
        s, d, key, counts = per_core[c]
        starts = np.zeros(NBLK * 2 + 1, np.int64)
        np.cumsum(counts, out=starts[1:])
        for g in range(NBLK * 2):
            n = counts[g]
            if n == 0:
                continue
            sl = slice(starts[g], starts[g] + n)
            h = g & 1
            idx_all[c, g, :n] = (s[sl] - h * half_b).astype(np.int16)
            dl_all[c, g, :n] = (d[sl] & 127).astype(np.float16)

    # wrap gather indices per (block, half) call: idx i -> [i%16, i//16],
    # replicated 8x to 128 rows (one copy per Q7 core)
    w0, w1 = kch0 * P // 16, kch1 * P // 16
    idx_w = np.zeros((NC, 128, NBLK * (w0 + w1)), np.int16)
    cpb = kch0 + kch1
    dl_T = np.full((NC, 128, NBLK * cpb), 200.0, np.float16)
    for c in range(NC):
        for b in range(NBLK):
            for h, (kch, woff) in enumerate(((kch0, 0), (kch1, w0))):
                part = idx_all[c, b * 2 + h, :kch * P]
                w = part.reshape(kch * P // 16, 16).T
                c0 = b * (w0 + w1) + woff
                idx_w[c, :, c0:c0 + kch * P // 16] = np.tile(w, (8, 1))
                dpart = dl_all[c, b * 2 + h, :kch * P].reshape(kch, P)
                ci0 = b * cpb + (0 if h == 0 else kch0)
                dl_T[c, :, ci0:ci0 + kch] = dpart.T
    return kchs, half_b, idx_w, dl_T


def _build(kchs, half_b, b1_any, b2_any):
    kch0, kch1 = kchs
    cpb = kch0 + kch1            # chunks per dst block
    nchunk = NBLK * cpb
    iw0, iw1 = kch0 * P // 16, kch1 * P // 16
    nc = bacc.Bacc(None, target_bir_lowering=False)

    # ---- inputs (per core) ----
    embT_in = nc.dram_tensor("embT", [EMB, NBLK * P], F16, kind="ExternalInput")
    w1_in = nc.dram_tensor("w1ext", [EMB, HIDDEN + 16], F16, kind="ExternalInput")
    w2_in = nc.dram_tensor("w2ext", [HIDDEN, REPR + 2], F16, kind="ExternalInput")
    idx_in = nc.dram_tensor("idxw", [128, NBLK * (iw0 + iw1)], I16, kind="ExternalInput")
    dl_in = nc.dram_tensor("dlT", [128, nchunk], F16, kind="ExternalInput")
    iota_in = nc.dram_tensor("iota", [P, P], F16, kind="ExternalInput")
    iotar_in = nc.dram_tensor("iotar", [P, cpb * P], F16, kind="ExternalInput")
    if b1_any:
        b1_in = nc.dram_tensor("b1e", [P, HIDDEN], F16, kind="ExternalInput")
    if b2_any:
        b2_in = nc.dram_tensor("b2e", [P, REPR], F16, kind="ExternalInput")
    out_t = nc.dram_tensor("out", [NSH, REPR], F32, kind="ExternalOutput")

    # ---- internal DRAM ----
    t1_shard = nc.dram_tensor("t1_shard", [NSH, T1_ELEM], F16, kind="Internal")
    t1_full = nc.dram_tensor("t1_full", [N, T1_ELEM], F16, kind="Internal",
                             addr_space="Shared")
    t2_shard = nc.dram_tensor("t2_shard", [NSH, T2_ELEM], F16, kind="Internal")
    t2_full = nc.dram_tensor("t2_full", [N, T2_ELEM], F16, kind="Internal",
                             addr_space="Shared")

    with tile.TileContext(nc) as tc:
        with (
            tc.tile_pool(name="const", bufs=1) as cst,
            tc.tile_pool(name="sb", bufs=3) as sb,
            tc.tile_pool(name="gp", bufs=6 if cpb <= 13 else 5) as gp,
            tc.tile_pool(name="rp", bufs=3) as rp,
            tc.tile_pool(name="sb2", bufs=3) as sb2,
            tc.tile_pool(name="oh", bufs=4 if cpb <= 13 else 3) as ohp,
            tc.tile_pool(name="psA", bufs=3, space="PSUM") as psA,
            tc.tile_pool(name="psB", bufs=2, space="PSUM") as psB,
            tc.tile_pool(name="psC", bufs=1, space="PSUM") as psC,
            tc.tile_pool(name="psD", bufs=2, space="PSUM") as psD,
        ):
            # ---- constants ----
            iota = cst.tile([P, P], F16)
            nc.sync.dma_start(out=iota[:], in_=iota_in[:])
            iotar = cst.tile([P, cpb * P], F16)
            nc.sync.dma_start(out=iotar[:], in_=iotar_in[:])
            ident = cst.tile([P, P], F16)
            make_identity(nc, ident[:])
            w1 = cst.tile([EMB, HIDDEN + 16], F16)
            nc.sync.dma_start(out=w1[:], in_=w1_in[:])
            w2 = cst.tile([P, 2, REPR + 2], F16)
            nc.sync.dma_start(out=w2[:, 0, :], in_=w2_in[0:P, :])
            nc.sync.dma_start(out=w2[:, 1, :], in_=w2_in[P:HIDDEN, :])
            it_all = cst.tile([128, NBLK * (iw0 + iw1)], I16)
            nc.sync.dma_start(out=it_all[:], in_=idx_in[:])
            dl_all = cst.tile([128, nchunk], F16)
            nc.sync.dma_start(out=dl_all[:], in_=dl_in[:])
            adst1 = cst.tile([P, NBLK * 8], F16)
            adst2 = cst.tile([P, NBLK], F16)
            if b1_any:
                b1e = cst.tile([P, HIDDEN], F16)
                nc.sync.dma_start(out=b1e[:], in_=b1_in[:])
            if b2_any:
                b2e = cst.tile([P, REPR], F16)
                nc.sync.dma_start(out=b2e[:], in_=b2_in[:])

            # ---- phase A: h1 GEMM + table build (4 blocks per DMA) ----
            GA = 4
            for b0 in range(0, NBLK, GA):
                nb = min(GA, NBLK - b0)
                et = sb.tile([EMB, GA, P], F16, tag="embT")
                nc.sync.dma_start(
                    out=et[:, 0:nb, :].rearrange("p a n -> p (a n)"),
                    in_=embT_in[:, b0 * P:(b0 + nb) * P])
                t1s = sb.tile([P, GA, T1_ELEM], F16, tag="t1s")
                for j in range(nb):
                    b = b0 + j
                    ph1 = psA.tile([P, HIDDEN + 16], F32, tag="acc")
                    nc.tensor.matmul(out=ph1[:], lhsT=et[:, j, :], rhs=w1[:],
                                     start=True, stop=True)
                    nc.vector.tensor_copy(out=t1s[:, j, 0:HIDDEN],
                                          in_=ph1[:, 0:HIDDEN])
                    nc.scalar.copy(out=t1s[:, j, HIDDEN:HIDDEN + 8],
                                   in_=ph1[:, HIDDEN:HIDDEN + 8])
                    nc.vector.tensor_copy(out=adst1[:, b * 8:(b + 1) * 8],
                                          in_=ph1[:, HIDDEN + 8:HIDDEN + 16])
                full = nb if b0 + nb < NBLK else nb - 1
                if full:
                    nc.sync.dma_start(
                        out=t1_shard[b0 * P:(b0 + full) * P, :]
                            .rearrange("(a p) e -> p a e", p=P),
                        in_=t1s[:, 0:full, :])
                if full < nb:
                    nc.sync.dma_start(
                        out=t1_shard[(b0 + full) * P:(b0 + full) * P + LASTB, :],
                        in_=t1s[:LASTB, full, :])

            nc.gpsimd.collective_compute(
                "AllGather", ALU.bypass, ins=[t1_shard[:]], outs=[t1_full[:]],
                replica_groups=[list(range(NC))])

            # ---- edge phase helper ----
            def edge_layer(t_full, elem, hid, heads, adst_t, out_cb):
                """One GAT message-passing layer over this core's dst blocks.

                t_full: gather table [N, elem] fp16, row = [feat(hid)|a_src(heads)|pad]
                adst_t: [P, NBLK*heads] per-block a_dst values
                out_cb(b, ps_acc): consume the accumulated [P, hid+heads] psum
                """
                mcols = hid + heads          # matmul rhs columns (msg | exp)
                PRE = 2                      # gather prefetch distance (blocks)
                gq = {}

                def issue_gathers(b):
                    gs = []
                    for h, (kch, woff, r0, r1) in enumerate((
                            (kch0, 0, 0, half_b), (kch1, iw0, half_b, N))):
                        gt = gp.tile([P, kch, elem], F16, tag=f"g{hid}{h}")
                        col0 = b * (iw0 + iw1) + woff
                        # <=7 chunks (896 idxs) per call: the SWDGE descriptor
                        # ring holds 1024
                        for off in range(0, kch, 7):
                            t = min(7, kch - off)
                            nc.gpsimd.dma_gather(
                                out_ap=gt[:, off:off + t, :],
                                in_ap=t_full[r0:r1, :],
                                idxs_ap=it_all[:, col0 + off * 8:
                                               col0 + (off + t) * 8],
                                num_idxs=t * P, num_idxs_reg=t * P,
                                elem_size=elem)
                        gs.append((gt[:], kch))
                    gq[b] = gs

                for b in range(min(PRE, NBLK)):
                    issue_gathers(b)
                for b in range(NBLK):
                    if b + PRE < NBLK:
                        issue_gathers(b + PRE)
                    gs = gq.pop(b)
                    # one-hots (one batched is_equal) + transposed one-hots
                    pse = psC.tile([P, cpb * heads], F32, tag="adst")
                    oh_all = ohp.tile([P, cpb, P], F16, tag="oh")
                    nc.vector.tensor_tensor(
                        out=oh_all[:],
                        in0=dl_all[:, b * cpb:(b + 1) * cpb]
                            .rearrange("p (t o) -> p t o", o=1)
                            .to_broadcast([P, cpb, P]),
                        in1=iotar[:].rearrange("p (t n) -> p t n", n=P),
                        op=ALU.is_equal)
                    ohs = [oh_all[:, k, :] for k in range(cpb)]
                    ohT_sb = ohp.tile([P, cpb, P], F16, tag="ohT_sb")
                    PSB_CH = 8   # chunks per fp16 psum bank
                    for g0 in range(0, cpb, PSB_CH):
                        g1 = min(g0 + PSB_CH, cpb)
                        pst = psB.tile([P, PSB_CH, P], F16, tag="ohT")
                        for k in range(g0, g1):
                            nc.tensor.transpose(out=pst[:, k - g0, :], in_=ohs[k],
                                                identity=ident[:])
                        nc.scalar.copy(out=ohT_sb[:, g0:g1, :].rearrange("p t n -> p (t n)"),
                                       in_=pst[:, 0:g1 - g0, :].rearrange("p t n -> p (t n)"))
                    for k in range(cpb):
                        nc.tensor.matmul(
                            out=pse[:, k * heads:(k + 1) * heads],
                            lhsT=ohT_sb[:, k, :],
                            rhs=adst_t[:, b * heads:(b + 1) * heads],
                            start=True, stop=True)
                    # logits -> leaky -> exp  (batched over the block's chunks)
                    e_sb = sb2.tile([P, cpb * heads], F32, tag=f"e{hid}")
                    k = 0
                    for gt, t in gs:
                        nc.vector.tensor_tensor(
                            out=e_sb[:, k * heads:(k + t) * heads]
                                .rearrange("p (t h) -> p t h", t=t),
                            in0=gt[:, :, hid:hid + heads],
                            in1=pse[:, k * heads:(k + t) * heads]
                                .rearrange("p (t h) -> p t h", t=t),
                            op=ALU.add)
                        k += t
                    lk = sb2.tile([P, cpb * heads], F32, tag=f"lk{hid}")
                    nc.scalar.activation(out=lk[:], in_=e_sb[:], func=AF.Prelu,
                                         alpha=NEG_SLOPE)
                    ex = sb2.tile([P, cpb * heads], F16, tag=f"ex{hid}")
                    nc.scalar.activation(out=ex[:], in_=lk[:], func=AF.Exp)
                    # messages (feat * exp, broadcast over feat/head) + exp col
                    rhs = rp.tile([P, cpb, mcols], F16, tag=f"rhs{hid}")
                    k = 0
                    for gi, (gt, t) in enumerate(gs):
                        # balance: route one L1 half's multiply to GPSIMD
                        eng = nc.gpsimd if (hid == HIDDEN and gi == 1) else nc.vector
                        eng.tensor_tensor(
                            out=rhs[:, k:k + t, 0:hid]
                                .rearrange("p t (h d) -> p t h d", h=heads),
                            in0=gt[:, :, 0:hid]
                                .rearrange("p t (h d) -> p t h d", h=heads),
                            in1=ex[:, k * heads:(k + t) * heads]
                                .rearrange("p (t h) -> p t h", t=t)[:, :, :, None]
                                .to_broadcast([P, t, heads, hid // heads]),
                            op=ALU.mult)
                        k += t
                    nc.vector.tensor_copy(
                        out=rhs[:, :, hid:hid + heads],
                        in_=ex[:].rearrange("p (t h) -> p t h", t=cpb))
                    # scatter-accumulate into the block's psum
                    pacc = psA.tile([P, mcols], F32, tag="acc")
                    for k in range(cpb):
                        nc.tensor.matmul(out=pacc[:], lhsT=ohs[k],
                                         rhs=rhs[:, k, :], start=(k == 0),
                                         stop=(k == cpb - 1))
                    out_cb(b, pacc)

            # ---- layer 1 block finisher: normalize, ELU, GEMM2, T2 rows ----
            def finish1(b, pacc):
                rows = P if b < NBLK - 1 else LASTB
                se = sb.tile([P, HEADS], F32, tag="se")
                nc.vector.tensor_scalar_add(out=se[:], in0=pacc[:, HIDDEN:HIDDEN + 8],
                                            scalar1=1e-16)
                rec = sb.tile([P, HEADS], F32, tag="rec")
                nc.vector.reciprocal(out=rec[:], in_=se[:])
                v = sb.tile([P, HIDDEN], F32, tag="v")
                nc.vector.tensor_tensor(
                    out=v[:].rearrange("p (h d) -> p h d", h=HEADS),
                    in0=pacc[:, 0:HIDDEN].rearrange("p (h d) -> p h d", h=HEADS),
                    in1=rec[:, :, None].to_broadcast([P, HEADS, OUT1]),
                    op=ALU.mult)
                if b1_any:
                    nc.vector.tensor_tensor(out=v[:], in0=v[:], in1=b1e[:], op=ALU.add)
                # elu(v) = relu(v) + exp(min(v,0)) - 1
                r = sb.tile([P, HIDDEN], F32, tag="relu")
                nc.scalar.activation(out=r[:], in_=v[:], func=AF.Relu)
                mn = sb.tile([P, HIDDEN], F32, tag="mn")
                nc.vector.tensor_scalar_min(out=mn[:], in0=v[:], scalar1=0.0)
                em = sb.tile([P, HIDDEN], F32, tag="em")
                nc.scalar.activation(out=em[:], in_=mn[:], func=AF.Exp)
                x = sb.tile([P, HIDDEN], F32, tag="x")
                nc.vector.tensor_tensor(out=x[:], in0=r[:], in1=em[:], op=ALU.add)
                x16 = sb.tile([P, HIDDEN], F16, tag="x16")
                nc.vector.tensor_scalar_add(out=x16[:], in0=x[:], scalar1=-1.0)
                # GEMM2: h2 = x @ W2ext  (transpose x tiles for lhsT)
                xT = sb.tile([P, 2, P], F16, tag="xT")
                for k in range(2):
                    pst = psD.tile([P, P], F16, tag="misc")
                    nc.tensor.transpose(out=pst[:], in_=x16[:, k * P:(k + 1) * P],
                                        identity=ident[:])
                    nc.scalar.copy(out=xT[:, k, :], in_=pst[:])
                ph2 = psD.tile([P, REPR + 2], F32, tag="misc")
                for k in range(2):
                    nc.tensor.matmul(out=ph2[:], lhsT=xT[:, k, :], rhs=w2[:, k, :],
                                     start=(k == 0), stop=(k == 1))
                t2s = sb.tile([P, T2_ELEM], F16, tag="t2s")
                nc.scalar.copy(out=t2s[:, 0:REPR], in_=ph2[:, 0:REPR])
                nc.vector.tensor_copy(out=t2s[:, REPR:REPR + 1],
                                      in_=ph2[:, REPR:REPR + 1])
                nc.vector.tensor_copy(out=adst2[:, b:b + 1],
                                      in_=ph2[:, REPR + 1:REPR + 2])
                nc.sync.dma_start(out=t2_shard[b * P:b * P + rows, :],
                                  in_=t2s[:rows, :])

            edge_layer(t1_full, T1_ELEM, HIDDEN, HEADS, adst1, finish1)

            nc.gpsimd.collective_compute(
                "AllGather", ALU.bypass, ins=[t2_shard[:]], outs=[t2_full[:]],
                replica_groups=[list(range(NC))])

            # ---- layer 2 block finisher: normalize -> output ----
            def finish2(b, pacc):
                rows = P if b < NBLK - 1 else LASTB
                se = sb.tile([P, 1], F32, tag="se2")
                nc.vector.tensor_scalar_add(out=se[:], in0=pacc[:, REPR:REPR + 1],
                                            scalar1=1e-16)
                rec = sb.tile([P, 1], F32, tag="rec2")
                nc.vector.reciprocal(out=rec[:], in_=se[:])
                o = sb.tile([P, REPR], F32, tag="o")
                nc.scalar.activation(out=o[:], in_=pacc[:, 0:REPR], func=AF.Copy,
                                     scale=rec[:, 0:1])
                if b2_any:
                    nc.vector.tensor_tensor(out=o[:], in0=o[:], in1=b2e[:], op=ALU.add)
                nc.sync.dma_start(out=out_t[b * P:b * P + rows, :], in_=o[:rows, :])

            edge_layer(t2_full, T2_ELEM, REPR, 1, adst2, finish2)

    nc.finalize()
    globals()["LAST_NC"] = nc
    return nc


def kernel(**inputs):
    node_emb = np.asarray(inputs["node_emb"], np.float32)
    W1 = np.asarray(inputs["W1"], np.float32)
    att1_src = np.asarray(inputs["att1_src"], np.float32)
    att1_dst = np.asarray(inputs["att1_dst"], np.float32)
    b1 = np.asarray(inputs["b1"], np.float32)
    W2 = np.asarray(inputs["W2"], np.float32)
    att2_src = np.asarray(inputs["att2_src"], np.float32)
    att2_dst = np.asarray(inputs["att2_dst"], np.float32)
    b2 = np.asarray(inputs["b2"], np.float32)
    edge_index = np.asarray(inputs["edge_index"])

    kchs, half_b, idx_w, dl_T = _prep_edges(edge_index)

    # fold attention projections into the GEMMs: a_src = emb @ (W1 . att)
    A1s = np.einsum("ehd,hd->eh", W1.reshape(EMB, HEADS, OUT1), att1_src)
    A1d = np.einsum("ehd,hd->eh", W1.reshape(EMB, HEADS, OUT1), att1_dst)
    w1ext = np.concatenate([W1, A1s, A1d], axis=1).astype(np.float16)
    A2s = W2 @ att2_src[0]
    A2d = W2 @ att2_dst[0]
    w2ext = np.concatenate([W2, A2s[:, None], A2d[:, None]], axis=1).astype(np.float16)

    iota = np.tile(np.arange(P, dtype=np.float16), (P, 1))
    iotar = np.tile(np.arange(P, dtype=np.float16), (P, sum(kchs)))
    b1_any = bool(np.any(b1))
    b2_any = bool(np.any(b2))

    nc = _build(kchs, half_b, b1_any, b2_any)

    embT_pad = np.zeros((NC, EMB, NBLK * P), np.float16)
    for c in range(NC):
        embT_pad[c, :, :NSH] = node_emb[c * NSH:(c + 1) * NSH].T.astype(np.float16)

    in_maps = []
    for c in range(NC):
        m = {
            "embT": embT_pad[c],
            "w1ext": w1ext,
            "w2ext": w2ext,
            "idxw": idx_w[c],
            "dlT": dl_T[c],
            "iota": iota,
            "iotar": iotar,
        }
        if b1_any:
            m["b1e"] = np.tile(b1[None, :], (P, 1)).astype(np.float16)
        if b2_any:
            m["b2e"] = np.tile(b2[None, :], (P, 1)).astype(np.float16)
        in_maps.append(m)

    res = run_bass_kernel_spmd(nc, in_maps, core_ids=list(range(NC)))
    out = np.concatenate([res.results[c]["out"] for c in range(NC)], axis=0)
    return np.ascontiguousarray(out.astype(np.float32))


if __name__ == "__main__":
    # quick self-exercise with random inputs of the right shapes
    rng = np.random.default_rng(0)
    ins = {
        "node_emb": rng.standard_normal((N, EMB), dtype=np.float32) * 0.05,
        "W1": rng.standard_normal((EMB, HIDDEN), dtype=np.float32) * 0.07,
        "att1_src": rng.standard_normal((HEADS, OUT1), dtype=np.float32) * 0.2,
        "att1_dst": rng.standard_normal((HEADS, OUT1), dtype=np.float32) * 0.2,
        "b1": np.zeros(HIDDEN, np.float32),
        "W2": rng.standard_normal((HIDDEN, REPR), dtype=np.float32) * 0.07,
        "att2_src": rng.standard_normal((1, REPR), dtype=np.float32) * 0.2,
        "att2_dst": rng.standard_normal((1, REPR), dtype=np.float32) * 0.2,
        "b2": np.zeros(REPR, np.float32),
        "edge_index": rng.integers(0, N, (2, E)).astype(np.int32),
    }
    out = kernel(**ins)
    print("out", out.shape, out.dtype, np.abs(out).mean())


# revision 33
# speedup vs baseline: 1.3473x; 1.0193x over previous
"""GAT (2-layer) on 8 Trainium2 NeuronCores — edge-parallel by destination.

Strategy
--------
- Nodes are sharded 8 ways (6250/core). Edges (incl. self-loops) are routed to
  the core that owns their destination node, so each core's scatter-adds are
  purely local (no all-reduce of aggregates).
- Phase A (per core): GEMM over its node shard computes h1 plus the attention
  logit projections a_src/a_dst (folded into the GEMM as extra output columns);
  rows [h1|a_src] are packed fp16 into a 768B-stride table, AllGathered so every
  core holds the full table.
- Edge phase (per core): edges are grouped by (128-node dst block, src-half) and
  chunked 128 at a time. Per chunk: dma_gather of the 128 source rows; a one-hot
  (edge x node) matrix built by is_equal against an iota constant; a_dst
  expanded edge-wise via a small matmul with the transposed one-hot; logits ->
  LeakyReLU(0.2) -> exp on the scalar engine; messages = h1_src * exp; the
  weighted scatter-sum is one fp16 matmul accumulating [128 nodes x (msg|exp)]
  in PSUM over the block (softmax denominator rides along as an extra column).
  Normalization (divide by the exp-sum) happens once per node at the end
  (softmax is shift-invariant; logits are O(1) so no max-subtraction needed).
- Between layers: ELU, second GEMM (x @ W2ext), second table, AllGather, same
  edge phase with 1 head, then the final normalize produces the output shard.
"""

import math
import os

import numpy as np

import concourse.bass as bass
import concourse.mybir as mybir
import concourse.tile as tile
from concourse import bacc
from concourse.bass_utils import run_bass_kernel_spmd
from concourse.masks import make_identity

# problem constants (from the reference)
N = 50000
E = 500000
EMB = 128
HIDDEN = 256
HEADS = 8
OUT1 = 32
REPR = 64
NEG_SLOPE = 0.2

NC = 8
P = 128
NSH = N // NC                    # 6250 nodes per core
NBLK = (NSH + P - 1) // P        # 49 dst blocks per core
LASTB = NSH - (NBLK - 1) * P     # 106 nodes in last block
HALF_B_CANDIDATES = range(25000, 28251, 250)   # src table split candidates
                                               # (both sides < 32768 rows)

T1_ELEM = 384                    # fp16: h1(256) | a_src(8) | pad -> 768B rows
T2_ELEM = 256                    # fp16: h2(64) | a2_src(1) | pad -> 512B rows

F16 = mybir.dt.float16
F32 = mybir.dt.float32
I16 = mybir.dt.int16
AF = mybir.ActivationFunctionType
ALU = mybir.AluOpType

MAX_CH_PER_CALL = 7              # 896 idxs/call, under the 1024-desc SWDGE ring


def _prep_edges(edge_index):
    """Partition + sort edges; build per-core gather-index / dst-local arrays."""
    ei = np.asarray(edge_index)
    src = np.concatenate([ei[0], np.arange(N, dtype=np.int64)]).astype(np.int64)
    dst = np.concatenate([ei[1], np.arange(N, dtype=np.int64)]).astype(np.int64)

    core = dst // NSH
    # pick the src-half boundary minimizing total chunks per block (padding)
    best = None
    for B in HALF_B_CANDIDATES:
        m0 = m1 = 1
        for c in range(NC):
            m = core == c
            s, d = src[m], dst[m] - c * NSH
            key = (d >> 7) * 2 + (s >= B)
            counts = np.bincount(key, minlength=NBLK * 2)
            m0 = max(m0, int(counts[0::2].max()))
            m1 = max(m1, int(counts[1::2].max()))
        k0, k1 = math.ceil(m0 / P), math.ceil(m1 / P)
        margin = min(k0 * P - m0, k1 * P - m1)
        cand = (k0 + k1, -margin, B, k0, k1)
        if best is None or cand < best:
            best = cand
    _, _, half_b, kch0, kch1 = best

    per_core = []
    for c in range(NC):
        m = core == c
        s, d = src[m], dst[m] - c * NSH
        key = (d >> 7) * 2 + (s >= half_b)
        order = np.argsort(key, kind="stable")
        s, d, key = s[order], d[order], key[order]
        counts = np.bincount(key, minlength=NBLK * 2)
        per_core.append((s, d, key, counts))
    kchs = (kch0, kch1)
    kmax = max(kch0, kch1)
    idx_all = np.zeros((NC, NBLK * 2, kmax * P), np.int16)
    dl_all = np.full((NC, NBLK * 2, kmax * P), 200.0, np.float16)
    for c in range(NC):
        s, d, key, counts = per_core[c]
        starts = np.zeros(NBLK * 2 + 1, np.int64)
        np.cumsum(counts, out=starts[1:])
        for g in range(NBLK * 2):
            n = counts[g]
            if n == 0:
                continue
            sl = slice(starts[g], starts[g] + n)
            h = g & 1
            idx_all[c, g, :n] = (s[sl] - h * half_b).astype(np.int16)
            dl_all[c, g, :n] = (d[sl] & 127).astype(np.float16)

    # wrap gather indices per (block, half) call: idx i -> [i%16, i//16],
    # replicated 8x to 128 rows (one copy per Q7 core)
    w0, w1 = kch0 * P // 16, kch1 * P // 16
    idx_w = np.zeros((NC, 128, NBLK * (w0 + w1)), np.int16)
    cpb = kch0 + kch1
    dl_T = np.full((NC, 128, NBLK * cpb), 200.0, np.float16)
    for c in range(NC):
        for b in range(NBLK):
            for h, (kch, woff) in enumerate(((kch0, 0), (kch1, w0))):
                part = idx_all[c, b * 2 + h, :kch * P]
                w = part.reshape(kch * P // 16, 16).T
                c0 = b * (w0 + w1) + woff
                idx_w[c, :, c0:c0 + kch * P // 16] = np.tile(w, (8, 1))
                dpart = dl_all[c, b * 2 + h, :kch * P].reshape(kch, P)
                ci0 = b * cpb + (0 if h == 0 else kch0)
                dl_T[c, :, ci0:ci0 + kch] = dpart.T
    return kchs, half_b, idx_w, dl_T


def _build(kchs, half_b, b1_any, b2_any):
    kch0, kch1 = kchs
    cpb = kch0 + kch1            # chunks per dst block
    nchunk = NBLK * cpb
    iw0, iw1 = kch0 * P // 16, kch1 * P // 16
    nc = bacc.Bacc(None, target_bir_lowering=False)

    # ---- inputs (per core) ----
    embT_in = nc.dram_tensor("embT", [EMB, NBLK * P], F16, kind="ExternalInput")
    w1_in = nc.dram_tensor("w1ext", [EMB, HIDDEN + 16], F16, kind="ExternalInput")
    w2_in = nc.dram_tensor("w2ext", [HIDDEN, REPR + 2], F16, kind="ExternalInput")
    idx_in = nc.dram_tensor("idxw", [128, NBLK * (iw0 + iw1)], I16, kind="ExternalInput")
    dl_in = nc.dram_tensor("dlT", [128, nchunk], F16, kind="ExternalInput")
    iota_in = nc.dram_tensor("iota", [P, P], F16, kind="ExternalInput")
    iotar_in = nc.dram_tensor("iotar", [P, cpb * P], F16, kind="ExternalInput")
    if b1_any:
        b1_in = nc.dram_tensor("b1e", [P, HIDDEN], F16, kind="ExternalInput")
    if b2_any:
        b2_in = nc.dram_tensor("b2e", [P, REPR], F16, kind="ExternalInput")
    out_t = nc.dram_tensor("out", [NSH, REPR], F32, kind="ExternalOutput")

    # ---- internal DRAM ----
    t1_shard = nc.dram_tensor("t1_shard", [NSH, T1_ELEM], F16, kind="Internal")
    t1_full = nc.dram_tensor("t1_full", [N, T1_ELEM], F16, kind="Internal",
                             addr_space="Shared")
    t2_shard = nc.dram_tensor("t2_shard", [NSH, T2_ELEM], F16, kind="Internal")
    t2_full = nc.dram_tensor("t2_full", [N, T2_ELEM], F16, kind="Internal",
                             addr_space="Shared")

    with tile.TileContext(nc) as tc:
        with (
            tc.tile_pool(name="const", bufs=1) as cst,
            tc.tile_pool(name="sb", bufs=3) as sb,
            tc.tile_pool(name="gp", bufs=6 if cpb <= 13 else 5) as gp,
            tc.tile_pool(name="rp", bufs=3) as rp,
            tc.tile_pool(name="sb2", bufs=3) as sb2,
            tc.tile_pool(name="oh", bufs=4 if cpb <= 13 else 3) as ohp,
            tc.tile_pool(name="psA", bufs=3, space="PSUM") as psA,
            tc.tile_pool(name="psB", bufs=2, space="PSUM") as psB,
            tc.tile_pool(name="psC", bufs=1, space="PSUM") as psC,
            tc.tile_pool(name="psD", bufs=2, space="PSUM") as psD,
        ):
            # ---- constants ----
            iota = cst.tile([P, P], F16)
            nc.sync.dma_start(out=iota[:], in_=iota_in[:])
            iotar = cst.tile([P, cpb * P], F16)
            nc.sync.dma_start(out=iotar[:], in_=iotar_in[:])
            ident = cst.tile([P, P], F16)
            make_identity(nc, ident[:])
            w1 = cst.tile([EMB, HIDDEN + 16], F16)
            nc.sync.dma_start(out=w1[:], in_=w1_in[:])
            w2 = cst.tile([P, 2, REPR + 2], F16)
            nc.sync.dma_start(out=w2[:, 0, :], in_=w2_in[0:P, :])
            nc.sync.dma_start(out=w2[:, 1, :], in_=w2_in[P:HIDDEN, :])
            it_all = cst.tile([128, NBLK * (iw0 + iw1)], I16)
            nc.sync.dma_start(out=it_all[:], in_=idx_in[:])
            dl_all = cst.tile([128, nchunk], F16)
            nc.sync.dma_start(out=dl_all[:], in_=dl_in[:])
            adst1 = cst.tile([P, NBLK * 8], F16)
            adst2 = cst.tile([P, NBLK], F16)
            if b1_any:
                b1e = cst.tile([P, HIDDEN], F16)
                nc.sync.dma_start(out=b1e[:], in_=b1_in[:])
            if b2_any:
                b2e = cst.tile([P, REPR], F16)
                nc.sync.dma_start(out=b2e[:], in_=b2_in[:])

            # ---- phase A: h1 GEMM + table build (4 blocks per DMA) ----
            GA = 4
            for b0 in range(0, NBLK, GA):
                nb = min(GA, NBLK - b0)
                et = sb.tile([EMB, GA, P], F16, tag="embT")
                nc.sync.dma_start(
                    out=et[:, 0:nb, :].rearrange("p a n -> p (a n)"),
                    in_=embT_in[:, b0 * P:(b0 + nb) * P])
                t1s = sb.tile([P, GA, T1_ELEM], F16, tag="t1s")
                for j in range(nb):
                    b = b0 + j
                    ph1 = psA.tile([P, HIDDEN + 16], F32, tag="acc")
                    nc.tensor.matmul(out=ph1[:], lhsT=et[:, j, :], rhs=w1[:],
                                     start=True, stop=True)
                    nc.vector.tensor_copy(out=t1s[:, j, 0:HIDDEN],
                                          in_=ph1[:, 0:HIDDEN])
                    nc.scalar.copy(out=t1s[:, j, HIDDEN:HIDDEN + 8],
                                   in_=ph1[:, HIDDEN:HIDDEN + 8])
                    nc.vector.tensor_copy(out=adst1[:, b * 8:(b + 1) * 8],
                                          in_=ph1[:, HIDDEN + 8:HIDDEN + 16])
                full = nb if b0 + nb < NBLK else nb - 1
                if full:
                    nc.sync.dma_start(
                        out=t1_shard[b0 * P:(b0 + full) * P, :]
                            .rearrange("(a p) e -> p a e", p=P),
                        in_=t1s[:, 0:full, :])
                if full < nb:
                    nc.sync.dma_start(
                        out=t1_shard[(b0 + full) * P:(b0 + full) * P + LASTB, :],
                        in_=t1s[:LASTB, full, :])

            nc.gpsimd.collective_compute(
                "AllGather", ALU.bypass, ins=[t1_shard[:]], outs=[t1_full[:]],
                replica_groups=[list(range(NC))])

            # ---- edge phase helper ----
            def edge_layer(t_full, elem, hid, heads, adst_t, out_cb):
                """One GAT message-passing layer over this core's dst blocks.

                t_full: gather table [N, elem] fp16, row = [feat(hid)|a_src(heads)|pad]
                adst_t: [P, NBLK*heads] per-block a_dst values
                out_cb(b, ps_acc): consume the accumulated [P, hid+heads] psum
                """
                mcols = hid + heads          # matmul rhs columns (msg | exp)
                PRE = 2                      # gather prefetch distance (blocks)
                gq = {}

                def issue_gathers(b):
                    gs = []
                    for h, (kch, woff, r0, r1) in enumerate((
                            (kch0, 0, 0, half_b), (kch1, iw0, half_b, N))):
                        gt = gp.tile([P, kch, elem], F16, tag=f"g{hid}{h}")
                        col0 = b * (iw0 + iw1) + woff
                        # <=7 chunks (896 idxs) per call: the SWDGE descriptor
                        # ring holds 1024
                        for off in range(0, kch, 7):
                            t = min(7, kch - off)
                            nc.gpsimd.dma_gather(
                                out_ap=gt[:, off:off + t, :],
                                in_ap=t_full[r0:r1, :],
                                idxs_ap=it_all[:, col0 + off * 8:
                                               col0 + (off + t) * 8],
                                num_idxs=t * P, num_idxs_reg=t * P,
                                elem_size=elem)
                        gs.append((gt[:], kch))
                    gq[b] = gs

                PREP = 1                     # prep-bundle prefetch (blocks)
                pq = {}

                def issue_prep(b):
                    oh_all = ohp.tile([P, cpb, P], F16, tag="oh")
                    nc.vector.tensor_tensor(
                        out=oh_all[:],
                        in0=dl_all[:, b * cpb:(b + 1) * cpb]
                            .rearrange("p (t o) -> p t o", o=1)
                            .to_broadcast([P, cpb, P]),
                        in1=iotar[:].rearrange("p (t n) -> p t n", n=P),
                        op=ALU.is_equal)
                    ohs = [oh_all[:, k, :] for k in range(cpb)]
                    ohT_sb = ohp.tile([P, cpb, P], F16, tag="ohT_sb")
                    PSB_CH = 8   # chunks per fp16 psum bank
                    for g0 in range(0, cpb, PSB_CH):
                        g1 = min(g0 + PSB_CH, cpb)
                        pst = psB.tile([P, PSB_CH, P], F16, tag="ohT")
                        for k in range(g0, g1):
                            nc.tensor.transpose(out=pst[:, k - g0, :], in_=ohs[k],
                                                identity=ident[:])
                        nc.scalar.copy(
                            out=ohT_sb[:, g0:g1, :].rearrange("p t n -> p (t n)"),
                            in_=pst[:, 0:g1 - g0, :].rearrange("p t n -> p (t n)"))
                    pq[b] = (ohs, ohT_sb)

                for b in range(min(PRE, NBLK)):
                    issue_gathers(b)
                for b in range(min(PREP, NBLK)):
                    issue_prep(b)
                for b in range(NBLK):
                    if b + PRE < NBLK:
                        issue_gathers(b + PRE)
                    if b + PREP < NBLK:
                        issue_prep(b + PREP)
                    gs = gq.pop(b)
                    # prep bundle (prefetched): one-hots + transposed one-hots
                    ohs, ohT_sb = pq.pop(b)
                    pse = psC.tile([P, cpb * heads], F32, tag="adst")
                    for k in range(cpb):
                        nc.tensor.matmul(
                            out=pse[:, k * heads:(k + 1) * heads],
                            lhsT=ohT_sb[:, k, :],
                            rhs=adst_t[:, b * heads:(b + 1) * heads],
                            start=True, stop=False)
                    # logits: accumulate gathered a_src onto the a_dst
                    # expansion in PSUM via identity matmuls (frees the DVE)
                    k = 0
                    for gt, t in gs:
                        for j in range(t):
                            nc.tensor.matmul(
                                out=pse[:, (k + j) * heads:(k + j + 1) * heads],
                                lhsT=ident[:],
                                rhs=gt[:, j, hid:hid + heads],
                                start=False, stop=True)
                        k += t
                    lk = sb2.tile([P, cpb * heads], F32, tag=f"lk{hid}")
                    nc.scalar.activation(out=lk[:], in_=pse[:], func=AF.Prelu,
                                         alpha=NEG_SLOPE)
                    ex = sb2.tile([P, cpb * heads], F16, tag=f"ex{hid}")
                    nc.scalar.activation(out=ex[:], in_=lk[:], func=AF.Exp)
                    # messages (feat * exp, broadcast over feat/head) + exp col
                    rhs = rp.tile([P, cpb, mcols], F16, tag=f"rhs{hid}")
                    k = 0
                    for gi, (gt, t) in enumerate(gs):
                        # balance: route one L1 half's multiply to GPSIMD
                        eng = nc.gpsimd if (hid == HIDDEN and gi == 1) else nc.vector
                        eng.tensor_tensor(
                            out=rhs[:, k:k + t, 0:hid]
                                .rearrange("p t (h d) -> p t h d", h=heads),
                            in0=gt[:, :, 0:hid]
                                .rearrange("p t (h d) -> p t h d", h=heads),
                            in1=ex[:, k * heads:(k + t) * heads]
                                .rearrange("p (t h) -> p t h", t=t)[:, :, :, None]
                                .to_broadcast([P, t, heads, hid // heads]),
                            op=ALU.mult)
                        k += t
                    nc.scalar.copy(
                        out=rhs[:, :, hid:hid + heads],
                        in_=ex[:].rearrange("p (t h) -> p t h", t=cpb))
                    # scatter-accumulate into the block's psum
                    pacc = psA.tile([P, mcols], F32, tag="acc")
                    for k in range(cpb):
                        nc.tensor.matmul(out=pacc[:], lhsT=ohs[k],
                                         rhs=rhs[:, k, :], start=(k == 0),
                                         stop=(k == cpb - 1))
                    out_cb(b, pacc)

            # ---- layer 1 block finisher: normalize, ELU, GEMM2, T2 rows ----
            def finish1(b, pacc):
                rows = P if b < NBLK - 1 else LASTB
                se = sb.tile([P, HEADS], F32, tag="se")
                nc.vector.tensor_scalar_add(out=se[:], in0=pacc[:, HIDDEN:HIDDEN + 8],
                                            scalar1=1e-16)
                rec = sb.tile([P, HEADS], F32, tag="rec")
                nc.vector.reciprocal(out=rec[:], in_=se[:])
                v = sb.tile([P, HIDDEN], F32, tag="v")
                nc.vector.tensor_tensor(
                    out=v[:].rearrange("p (h d) -> p h d", h=HEADS),
                    in0=pacc[:, 0:HIDDEN].rearrange("p (h d) -> p h d", h=HEADS),
                    in1=rec[:, :, None].to_broadcast([P, HEADS, OUT1]),
                    op=ALU.mult)
                if b1_any:
                    nc.vector.tensor_tensor(out=v[:], in0=v[:], in1=b1e[:], op=ALU.add)
                # elu(v) = relu(v) + exp(min(v,0)) - 1
                r = sb.tile([P, HIDDEN], F32, tag="relu")
                nc.scalar.activation(out=r[:], in_=v[:], func=AF.Relu)
                mn = sb.tile([P, HIDDEN], F32, tag="mn")
                nc.scalar.activation(out=mn[:], in_=v[:], func=AF.Relu, scale=-1.0)
                em = sb.tile([P, HIDDEN], F32, tag="em")
                nc.scalar.activation(out=em[:], in_=mn[:], func=AF.Exp, scale=-1.0)
                x = sb.tile([P, HIDDEN], F32, tag="x")
                nc.vector.tensor_tensor(out=x[:], in0=r[:], in1=em[:], op=ALU.add)
                x16 = sb.tile([P, HIDDEN], F16, tag="x16")
                nc.vector.tensor_scalar_add(out=x16[:], in0=x[:], scalar1=-1.0)
                # GEMM2: h2 = x @ W2ext  (transpose x tiles for lhsT)
                xT = sb.tile([P, 2, P], F16, tag="xT")
                for k in range(2):
                    pst = psD.tile([P, P], F16, tag="misc")
                    nc.tensor.transpose(out=pst[:], in_=x16[:, k * P:(k + 1) * P],
                                        identity=ident[:])
                    nc.scalar.copy(out=xT[:, k, :], in_=pst[:])
                ph2 = psD.tile([P, REPR + 2], F32, tag="misc")
                for k in range(2):
                    nc.tensor.matmul(out=ph2[:], lhsT=xT[:, k, :], rhs=w2[:, k, :],
                                     start=(k == 0), stop=(k == 1))
                t2s = sb.tile([P, T2_ELEM], F16, tag="t2s")
                nc.scalar.copy(out=t2s[:, 0:REPR], in_=ph2[:, 0:REPR])
                nc.vector.tensor_copy(out=t2s[:, REPR:REPR + 1],
                                      in_=ph2[:, REPR:REPR + 1])
                nc.vector.tensor_copy(out=adst2[:, b:b + 1],
                                      in_=ph2[:, REPR + 1:REPR + 2])
                nc.sync.dma_start(out=t2_shard[b * P:b * P + rows, :],
                                  in_=t2s[:rows, :])

            edge_layer(t1_full, T1_ELEM, HIDDEN, HEADS, adst1, finish1)

            nc.gpsimd.collective_compute(
                "AllGather", ALU.bypass, ins=[t2_shard[:]], outs=[t2_full[:]],
                replica_groups=[list(range(NC))])

            # ---- layer 2 block finisher: normalize -> output ----
            def finish2(b, pacc):
                rows = P if b < NBLK - 1 else LASTB
                se = sb.tile([P, 1], F32, tag="se2")
                nc.vector.tensor_scalar_add(out=se[:], in0=pacc[:, REPR:REPR + 1],
                                            scalar1=1e-16)
                rec = sb.tile([P, 1], F32, tag="rec2")
                nc.vector.reciprocal(out=rec[:], in_=se[:])
                o = sb.tile([P, REPR], F32, tag="o")
                nc.scalar.activation(out=o[:], in_=pacc[:, 0:REPR], func=AF.Copy,
                                     scale=rec[:, 0:1])
                if b2_any:
                    nc.vector.tensor_tensor(out=o[:], in0=o[:], in1=b2e[:], op=ALU.add)
                nc.sync.dma_start(out=out_t[b * P:b * P + rows, :], in_=o[:rows, :])

            edge_layer(t2_full, T2_ELEM, REPR, 1, adst2, finish2)

    nc.finalize()
    globals()["LAST_NC"] = nc
    return nc


def kernel(**inputs):
    node_emb = np.asarray(inputs["node_emb"], np.float32)
    W1 = np.asarray(inputs["W1"], np.float32)
    att1_src = np.asarray(inputs["att1_src"], np.float32)
    att1_dst = np.asarray(inputs["att1_dst"], np.float32)
    b1 = np.asarray(inputs["b1"], np.float32)
    W2 = np.asarray(inputs["W2"], np.float32)
    att2_src = np.asarray(inputs["att2_src"], np.float32)
    att2_dst = np.asarray(inputs["att2_dst"], np.float32)
    b2 = np.asarray(inputs["b2"], np.float32)
    edge_index = np.asarray(inputs["edge_index"])

    kchs, half_b, idx_w, dl_T = _prep_edges(edge_index)

    # fold attention projections into the GEMMs: a_src = emb @ (W1 . att)
    A1s = np.einsum("ehd,hd->eh", W1.reshape(EMB, HEADS, OUT1), att1_src)
    A1d = np.einsum("ehd,hd->eh", W1.reshape(EMB, HEADS, OUT1), att1_dst)
    w1ext = np.concatenate([W1, A1s, A1d], axis=1).astype(np.float16)
    A2s = W2 @ att2_src[0]
    A2d = W2 @ att2_dst[0]
    w2ext = np.concatenate([W2, A2s[:, None], A2d[:, None]], axis=1).astype(np.float16)

    iota = np.tile(np.arange(P, dtype=np.float16), (P, 1))
    iotar = np.tile(np.arange(P, dtype=np.float16), (P, sum(kchs)))
    b1_any = bool(np.any(b1))
    b2_any = bool(np.any(b2))

    nc = _build(kchs, half_b, b1_any, b2_any)

    embT_pad = np.zeros((NC, EMB, NBLK * P), np.float16)
    for c in range(NC):
        embT_pad[c, :, :NSH] = node_emb[c * NSH:(c + 1) * NSH].T.astype(np.float16)

    in_maps = []
    for c in range(NC):
        m = {
            "embT": embT_pad[c],
            "w1ext": w1ext,
            "w2ext": w2ext,
            "idxw": idx_w[c],
            "dlT": dl_T[c],
            "iota": iota,
            "iotar": iotar,
        }
        if b1_any:
            m["b1e"] = np.tile(b1[None, :], (P, 1)).astype(np.float16)
        if b2_any:
            m["b2e"] = np.tile(b2[None, :], (P, 1)).astype(np.float16)
        in_maps.append(m)

    res = run_bass_kernel_spmd(nc, in_maps, core_ids=list(range(NC)))
    out = np.concatenate([res.results[c]["out"] for c in range(NC)], axis=0)
    return np.ascontiguousarray(out.astype(np.float32))


if __name__ == "__main__":
    # quick self-exercise with random inputs of the right shapes
    rng = np.random.default_rng(0)
    ins = {
        "node_emb": rng.standard_normal((N, EMB), dtype=np.float32) * 0.05,
        "W1": rng.standard_normal((EMB, HIDDEN), dtype=np.float32) * 0.07,
        "att1_src": rng.standard_normal((HEADS, OUT1), dtype=np.float32) * 0.2,
        "att1_dst": rng.standard_normal((HEADS, OUT1), dtype=np.float32) * 0.2,
        "b1": np.zeros(HIDDEN, np.float32),
        "W2": rng.standard_normal((HIDDEN, REPR), dtype=np.float32) * 0.07,
        "att2_src": rng.standard_normal((1, REPR), dtype=np.float32) * 0.2,
        "att2_dst": rng.standard_normal((1, REPR), dtype=np.float32) * 0.2,
        "b2": np.zeros(REPR, np.float32),
        "edge_index": rng.integers(0, N, (2, E)).astype(np.int32),
    }
    out = kernel(**ins)
    print("out", out.shape, out.dtype, np.abs(out).mean())


# revision 34
# speedup vs baseline: 1.3492x; 1.0014x over previous
"""GAT (2-layer) on 8 Trainium2 NeuronCores — edge-parallel by destination.

Strategy
--------
- Nodes are sharded 8 ways (6250/core). Edges (incl. self-loops) are routed to
  the core that owns their destination node, so each core's scatter-adds are
  purely local (no all-reduce of aggregates).
- Phase A (per core): GEMM over its node shard computes h1 plus the attention
  logit projections a_src/a_dst (folded into the GEMM as extra output columns);
  rows [h1|a_src] are packed fp16 into a 768B-stride table, AllGathered so every
  core holds the full table.
- Edge phase (per core): edges are grouped by (128-node dst block, src-half) and
  chunked 128 at a time. Per chunk: dma_gather of the 128 source rows; a one-hot
  (edge x node) matrix built by is_equal against an iota constant; a_dst
  expanded edge-wise via a small matmul with the transposed one-hot; logits ->
  LeakyReLU(0.2) -> exp on the scalar engine; messages = h1_src * exp; the
  weighted scatter-sum is one fp16 matmul accumulating [128 nodes x (msg|exp)]
  in PSUM over the block (softmax denominator rides along as an extra column).
  Normalization (divide by the exp-sum) happens once per node at the end
  (softmax is shift-invariant; logits are O(1) so no max-subtraction needed).
- Between layers: ELU, second GEMM (x @ W2ext), second table, AllGather, same
  edge phase with 1 head, then the final normalize produces the output shard.
"""

import math
import os

import numpy as np

import concourse.bass as bass
import concourse.mybir as mybir
import concourse.tile as tile
from concourse import bacc
from concourse.bass_utils import run_bass_kernel_spmd
from concourse.masks import make_identity

# problem constants (from the reference)
N = 50000
E = 500000
EMB = 128
HIDDEN = 256
HEADS = 8
OUT1 = 32
REPR = 64
NEG_SLOPE = 0.2

NC = 8
P = 128
NSH = N // NC                    # 6250 nodes per core
NBLK = (NSH + P - 1) // P        # 49 dst blocks per core
LASTB = NSH - (NBLK - 1) * P     # 106 nodes in last block
HALF_B_CANDIDATES = range(25000, 28251, 250)   # src table split candidates
                                               # (both sides < 32768 rows)

T1_ELEM = 384                    # fp16: h1(256) | a_src(8) | pad -> 768B rows
T2_ELEM = 256                    # fp16: h2(64) | a2_src(1) | pad -> 512B rows

F16 = mybir.dt.float16
F32 = mybir.dt.float32
I16 = mybir.dt.int16
AF = mybir.ActivationFunctionType
ALU = mybir.AluOpType

MAX_CH_PER_CALL = 7              # 896 idxs/call, under the 1024-desc SWDGE ring


def _prep_edges(edge_index):
    """Partition + sort edges; build per-core gather-index / dst-local arrays."""
    ei = np.asarray(edge_index)
    src = np.concatenate([ei[0], np.arange(N, dtype=np.int64)]).astype(np.int64)
    dst = np.concatenate([ei[1], np.arange(N, dtype=np.int64)]).astype(np.int64)

    core = dst // NSH
    # pick the src-half boundary minimizing total chunks per block (padding)
    best = None
    for B in HALF_B_CANDIDATES:
        m0 = m1 = 1
        for c in range(NC):
            m = core == c
            s, d = src[m], dst[m] - c * NSH
            key = (d >> 7) * 2 + (s >= B)
            counts = np.bincount(key, minlength=NBLK * 2)
            m0 = max(m0, int(counts[0::2].max()))
            m1 = max(m1, int(counts[1::2].max()))
        k0, k1 = math.ceil(m0 / P), math.ceil(m1 / P)
        margin = min(k0 * P - m0, k1 * P - m1)
        cand = (k0 + k1, -margin, B, k0, k1)
        if best is None or cand < best:
            best = cand
    _, _, half_b, kch0, kch1 = best

    per_core = []
    for c in range(NC):
        m = core == c
        s, d = src[m], dst[m] - c * NSH
        key = (d >> 7) * 2 + (s >= half_b)
        order = np.argsort(key, kind="stable")
        s, d, key = s[order], d[order], key[order]
        counts = np.bincount(key, minlength=NBLK * 2)
        per_core.append((s, d, key, counts))
    kchs = (kch0, kch1)
    kmax = max(kch0, kch1)
    idx_all = np.zeros((NC, NBLK * 2, kmax * P), np.int16)
    dl_all = np.full((NC, NBLK * 2, kmax * P), 200.0, np.float16)
    for c in range(NC):
        s, d, key, counts = per_core[c]
        starts = np.zeros(NBLK * 2 + 1, np.int64)
        np.cumsum(counts, out=starts[1:])
        for g in range(NBLK * 2):
            n = counts[g]
            if n == 0:
                continue
            sl = slice(starts[g], starts[g] + n)
            h = g & 1
            idx_all[c, g, :n] = (s[sl] - h * half_b).astype(np.int16)
            dl_all[c, g, :n] = (d[sl] & 127).astype(np.float16)

    # wrap gather indices per (block, half) call: idx i -> [i%16, i//16],
    # replicated 8x to 128 rows (one copy per Q7 core)
    w0, w1 = kch0 * P // 16, kch1 * P // 16
    idx_w = np.zeros((NC, 128, NBLK * (w0 + w1)), np.int16)
    cpb = kch0 + kch1
    dl_T = np.full((NC, 128, NBLK * cpb), 200.0, np.float16)
    for c in range(NC):
        for b in range(NBLK):
            for h, (kch, woff) in enumerate(((kch0, 0), (kch1, w0))):
                part = idx_all[c, b * 2 + h, :kch * P]
                w = part.reshape(kch * P // 16, 16).T
                c0 = b * (w0 + w1) + woff
                idx_w[c, :, c0:c0 + kch * P // 16] = np.tile(w, (8, 1))
                dpart = dl_all[c, b * 2 + h, :kch * P].reshape(kch, P)
                ci0 = b * cpb + (0 if h == 0 else kch0)
                dl_T[c, :, ci0:ci0 + kch] = dpart.T
    return kchs, half_b, idx_w, dl_T


def _build(kchs, half_b, b1_any, b2_any):
    kch0, kch1 = kchs
    cpb = kch0 + kch1            # chunks per dst block
    nchunk = NBLK * cpb
    iw0, iw1 = kch0 * P // 16, kch1 * P // 16
    nc = bacc.Bacc(None, target_bir_lowering=False)

    # ---- inputs (per core) ----
    embT_in = nc.dram_tensor("embT", [EMB, NBLK * P], F16, kind="ExternalInput")
    w1_in = nc.dram_tensor("w1ext", [EMB, HIDDEN + 16], F16, kind="ExternalInput")
    w2_in = nc.dram_tensor("w2ext", [HIDDEN, REPR + 2], F16, kind="ExternalInput")
    idx_in = nc.dram_tensor("idxw", [128, NBLK * (iw0 + iw1)], I16, kind="ExternalInput")
    dl_in = nc.dram_tensor("dlT", [128, nchunk], F16, kind="ExternalInput")
    iota_in = nc.dram_tensor("iota", [P, P], F16, kind="ExternalInput")
    iotar_in = nc.dram_tensor("iotar", [P, cpb * P], F16, kind="ExternalInput")
    if b1_any:
        b1_in = nc.dram_tensor("b1e", [P, HIDDEN], F16, kind="ExternalInput")
    if b2_any:
        b2_in = nc.dram_tensor("b2e", [P, REPR], F16, kind="ExternalInput")
    out_t = nc.dram_tensor("out", [NSH, REPR], F32, kind="ExternalOutput")

    # ---- internal DRAM ----
    t1_shard = nc.dram_tensor("t1_shard", [NSH, T1_ELEM], F16, kind="Internal")
    t1_full = nc.dram_tensor("t1_full", [N, T1_ELEM], F16, kind="Internal",
                             addr_space="Shared")
    t2_shard = nc.dram_tensor("t2_shard", [NSH, T2_ELEM], F16, kind="Internal")
    t2_full = nc.dram_tensor("t2_full", [N, T2_ELEM], F16, kind="Internal",
                             addr_space="Shared")

    with tile.TileContext(nc) as tc:
        with (
            tc.tile_pool(name="const", bufs=1) as cst,
            tc.tile_pool(name="sb", bufs=3) as sb,
            tc.tile_pool(name="gp", bufs=6 if cpb <= 13 else 5) as gp,
            tc.tile_pool(name="rp", bufs=3) as rp,
            tc.tile_pool(name="sb2", bufs=3) as sb2,
            tc.tile_pool(name="oh", bufs=4 if cpb <= 13 else 3) as ohp,
            tc.tile_pool(name="psA", bufs=3, space="PSUM") as psA,
            tc.tile_pool(name="psB", bufs=2, space="PSUM") as psB,
            tc.tile_pool(name="psC", bufs=1, space="PSUM") as psC,
            tc.tile_pool(name="psD", bufs=2, space="PSUM") as psD,
        ):
            # ---- constants ----
            iota = cst.tile([P, P], F16)
            nc.sync.dma_start(out=iota[:], in_=iota_in[:])
            iotar = cst.tile([P, cpb * P], F16)
            nc.sync.dma_start(out=iotar[:], in_=iotar_in[:])
            ident = cst.tile([P, P], F16)
            make_identity(nc, ident[:])
            w1 = cst.tile([EMB, HIDDEN + 16], F16)
            nc.sync.dma_start(out=w1[:], in_=w1_in[:])
            w2 = cst.tile([P, 2, REPR + 2], F16)
            nc.sync.dma_start(out=w2[:, 0, :], in_=w2_in[0:P, :])
            nc.sync.dma_start(out=w2[:, 1, :], in_=w2_in[P:HIDDEN, :])
            it_all = cst.tile([128, NBLK * (iw0 + iw1)], I16)
            nc.sync.dma_start(out=it_all[:], in_=idx_in[:])
            dl_all = cst.tile([128, nchunk], F16)
            nc.sync.dma_start(out=dl_all[:], in_=dl_in[:])
            adst1 = cst.tile([P, NBLK * 8], F16)
            adst2 = cst.tile([P, NBLK], F16)
            if b1_any:
                b1e = cst.tile([P, HIDDEN], F16)
                nc.sync.dma_start(out=b1e[:], in_=b1_in[:])
            if b2_any:
                b2e = cst.tile([P, REPR], F16)
                nc.sync.dma_start(out=b2e[:], in_=b2_in[:])

            # ---- phase A: h1 GEMM + table build (4 blocks per DMA) ----
            GA = 4
            for b0 in range(0, NBLK, GA):
                nb = min(GA, NBLK - b0)
                et = sb.tile([EMB, GA, P], F16, tag="embT")
                nc.sync.dma_start(
                    out=et[:, 0:nb, :].rearrange("p a n -> p (a n)"),
                    in_=embT_in[:, b0 * P:(b0 + nb) * P])
                t1s = sb.tile([P, GA, T1_ELEM], F16, tag="t1s")
                for j in range(nb):
                    b = b0 + j
                    ph1 = psA.tile([P, HIDDEN + 16], F32, tag="acc")
                    nc.tensor.matmul(out=ph1[:], lhsT=et[:, j, :], rhs=w1[:],
                                     start=True, stop=True)
                    nc.vector.tensor_copy(out=t1s[:, j, 0:HIDDEN],
                                          in_=ph1[:, 0:HIDDEN])
                    nc.scalar.copy(out=t1s[:, j, HIDDEN:HIDDEN + 8],
                                   in_=ph1[:, HIDDEN:HIDDEN + 8])
                    nc.vector.tensor_copy(out=adst1[:, b * 8:(b + 1) * 8],
                                          in_=ph1[:, HIDDEN + 8:HIDDEN + 16])
                full = nb if b0 + nb < NBLK else nb - 1
                if full:
                    nc.sync.dma_start(
                        out=t1_shard[b0 * P:(b0 + full) * P, :]
                            .rearrange("(a p) e -> p a e", p=P),
                        in_=t1s[:, 0:full, :])
                if full < nb:
                    nc.sync.dma_start(
                        out=t1_shard[(b0 + full) * P:(b0 + full) * P + LASTB, :],
                        in_=t1s[:LASTB, full, :])

            nc.gpsimd.collective_compute(
                "AllGather", ALU.bypass, ins=[t1_shard[:]], outs=[t1_full[:]],
                replica_groups=[list(range(NC))])

            # ---- edge phase helper ----
            def edge_layer(t_full, elem, hid, heads, adst_t, out_cb):
                """One GAT message-passing layer over this core's dst blocks.

                t_full: gather table [N, elem] fp16, row = [feat(hid)|a_src(heads)|pad]
                adst_t: [P, NBLK*heads] per-block a_dst values
                out_cb(b, ps_acc): consume the accumulated [P, hid+heads] psum
                """
                mcols = hid + heads          # matmul rhs columns (msg | exp)
                PRE = 2                      # gather prefetch distance (blocks)
                gq = {}

                def issue_gathers(b):
                    gs = []
                    for h, (kch, woff, r0, r1) in enumerate((
                            (kch0, 0, 0, half_b), (kch1, iw0, half_b, N))):
                        gt = gp.tile([P, kch, elem], F16, tag=f"g{hid}{h}")
                        col0 = b * (iw0 + iw1) + woff
                        # <=7 chunks (896 idxs) per call: the SWDGE descriptor
                        # ring holds 1024
                        for off in range(0, kch, 7):
                            t = min(7, kch - off)
                            nc.gpsimd.dma_gather(
                                out_ap=gt[:, off:off + t, :],
                                in_ap=t_full[r0:r1, :],
                                idxs_ap=it_all[:, col0 + off * 8:
                                               col0 + (off + t) * 8],
                                num_idxs=t * P, num_idxs_reg=t * P,
                                elem_size=elem)
                        gs.append((gt[:], kch))
                    gq[b] = gs

                PREP = 1                     # prep-bundle prefetch (blocks)
                pq = {}

                def issue_prep(b):
                    oh_all = ohp.tile([P, cpb, P], F16, tag="oh")
                    nc.vector.tensor_tensor(
                        out=oh_all[:],
                        in0=dl_all[:, b * cpb:(b + 1) * cpb]
                            .rearrange("p (t o) -> p t o", o=1)
                            .to_broadcast([P, cpb, P]),
                        in1=iotar[:].rearrange("p (t n) -> p t n", n=P),
                        op=ALU.is_equal)
                    ohs = [oh_all[:, k, :] for k in range(cpb)]
                    ohT_sb = ohp.tile([P, cpb, P], F16, tag="ohT_sb")
                    PSB_CH = 8   # chunks per fp16 psum bank
                    for g0 in range(0, cpb, PSB_CH):
                        g1 = min(g0 + PSB_CH, cpb)
                        pst = psB.tile([P, PSB_CH, P], F16, tag="ohT")
                        for k in range(g0, g1):
                            nc.tensor.transpose(out=pst[:, k - g0, :], in_=ohs[k],
                                                identity=ident[:])
                        nc.scalar.copy(
                            out=ohT_sb[:, g0:g1, :].rearrange("p t n -> p (t n)"),
                            in_=pst[:, 0:g1 - g0, :].rearrange("p t n -> p (t n)"))
                    pq[b] = (ohs, ohT_sb)

                for b in range(min(PRE, NBLK)):
                    issue_gathers(b)
                for b in range(min(PREP, NBLK)):
                    issue_prep(b)
                for b in range(NBLK):
                    if b + PRE < NBLK:
                        issue_gathers(b + PRE)
                    if b + PREP < NBLK:
                        issue_prep(b + PREP)
                    gs = gq.pop(b)
                    # prep bundle (prefetched): one-hots + transposed one-hots
                    ohs, ohT_sb = pq.pop(b)
                    pse = psC.tile([P, cpb * heads], F32, tag="adst")
                    def a_src_rhs(k):
                        off = k if k < gs[0][1] else k - gs[0][1]
                        return gs[0][0] if k < gs[0][1] else gs[1][0], off

                    for k in range(cpb):
                        nc.tensor.matmul(
                            out=pse[:, k * heads:(k + 1) * heads],
                            lhsT=ohT_sb[:, k, :],
                            rhs=adst_t[:, b * heads:(b + 1) * heads],
                            start=True, stop=False)
                        gtile, off = a_src_rhs(k)
                        nc.tensor.matmul(
                            out=pse[:, k * heads:(k + 1) * heads],
                            lhsT=ident[:],
                            rhs=gtile[:, off, hid:hid + heads],
                            start=False, stop=True)
                    lk = sb2.tile([P, cpb * heads], F32, tag=f"lk{hid}")
                    nc.scalar.activation(out=lk[:], in_=pse[:], func=AF.Prelu,
                                         alpha=NEG_SLOPE)
                    ex = sb2.tile([P, cpb * heads], F16, tag=f"ex{hid}")
                    nc.scalar.activation(out=ex[:], in_=lk[:], func=AF.Exp)
                    # messages (feat * exp, broadcast over feat/head) + exp col
                    rhs = rp.tile([P, cpb, mcols], F16, tag=f"rhs{hid}")
                    k = 0
                    for gi, (gt, t) in enumerate(gs):
                        # balance: route one L1 half's multiply to GPSIMD
                        eng = nc.gpsimd if (hid == HIDDEN and gi == 1) else nc.vector
                        eng.tensor_tensor(
                            out=rhs[:, k:k + t, 0:hid]
                                .rearrange("p t (h d) -> p t h d", h=heads),
                            in0=gt[:, :, 0:hid]
                                .rearrange("p t (h d) -> p t h d", h=heads),
                            in1=ex[:, k * heads:(k + t) * heads]
                                .rearrange("p (t h) -> p t h", t=t)[:, :, :, None]
                                .to_broadcast([P, t, heads, hid // heads]),
                            op=ALU.mult)
                        k += t
                    nc.scalar.copy(
                        out=rhs[:, :, hid:hid + heads],
                        in_=ex[:].rearrange("p (t h) -> p t h", t=cpb))
                    # scatter-accumulate into the block's psum
                    pacc = psA.tile([P, mcols], F32, tag="acc")
                    for k in range(cpb):
                        nc.tensor.matmul(out=pacc[:], lhsT=ohs[k],
                                         rhs=rhs[:, k, :], start=(k == 0),
                                         stop=(k == cpb - 1))
                    out_cb(b, pacc)

            # ---- layer 1 block finisher: normalize, ELU, GEMM2, T2 rows ----
            def finish1(b, pacc):
                rows = P if b < NBLK - 1 else LASTB
                se = sb.tile([P, HEADS], F32, tag="se")
                nc.vector.tensor_scalar_add(out=se[:], in0=pacc[:, HIDDEN:HIDDEN + 8],
                                            scalar1=1e-16)
                rec = sb.tile([P, HEADS], F32, tag="rec")
                nc.vector.reciprocal(out=rec[:], in_=se[:])
                v = sb.tile([P, HIDDEN], F32, tag="v")
                nc.vector.tensor_tensor(
                    out=v[:].rearrange("p (h d) -> p h d", h=HEADS),
                    in0=pacc[:, 0:HIDDEN].rearrange("p (h d) -> p h d", h=HEADS),
                    in1=rec[:, :, None].to_broadcast([P, HEADS, OUT1]),
                    op=ALU.mult)
                if b1_any:
                    nc.vector.tensor_tensor(out=v[:], in0=v[:], in1=b1e[:], op=ALU.add)
                # elu(v) = relu(v) + exp(min(v,0)) - 1
                r = sb.tile([P, HIDDEN], F32, tag="relu")
                nc.scalar.activation(out=r[:], in_=v[:], func=AF.Relu)
                mn = sb.tile([P, HIDDEN], F32, tag="mn")
                nc.scalar.activation(out=mn[:], in_=v[:], func=AF.Relu, scale=-1.0)
                em = sb.tile([P, HIDDEN], F32, tag="em")
                nc.scalar.activation(out=em[:], in_=mn[:], func=AF.Exp, scale=-1.0)
                x = sb.tile([P, HIDDEN], F32, tag="x")
                nc.vector.tensor_tensor(out=x[:], in0=r[:], in1=em[:], op=ALU.add)
                x16 = sb.tile([P, HIDDEN], F16, tag="x16")
                nc.vector.tensor_scalar_add(out=x16[:], in0=x[:], scalar1=-1.0)
                # GEMM2: h2 = x @ W2ext  (transpose x tiles for lhsT)
                xT = sb.tile([P, 2, P], F16, tag="xT")
                for k in range(2):
                    pst = psD.tile([P, P], F16, tag="misc")
                    nc.tensor.transpose(out=pst[:], in_=x16[:, k * P:(k + 1) * P],
                                        identity=ident[:])
                    nc.scalar.copy(out=xT[:, k, :], in_=pst[:])
                ph2 = psD.tile([P, REPR + 2], F32, tag="misc")
                for k in range(2):
                    nc.tensor.matmul(out=ph2[:], lhsT=xT[:, k, :], rhs=w2[:, k, :],
                                     start=(k == 0), stop=(k == 1))
                t2s = sb.tile([P, T2_ELEM], F16, tag="t2s")
                nc.scalar.copy(out=t2s[:, 0:REPR], in_=ph2[:, 0:REPR])
                nc.vector.tensor_copy(out=t2s[:, REPR:REPR + 1],
                                      in_=ph2[:, REPR:REPR + 1])
                nc.vector.tensor_copy(out=adst2[:, b:b + 1],
                                      in_=ph2[:, REPR + 1:REPR + 2])
                nc.sync.dma_start(out=t2_shard[b * P:b * P + rows, :],
                                  in_=t2s[:rows, :])

            edge_layer(t1_full, T1_ELEM, HIDDEN, HEADS, adst1, finish1)

            nc.gpsimd.collective_compute(
                "AllGather", ALU.bypass, ins=[t2_shard[:]], outs=[t2_full[:]],
                replica_groups=[list(range(NC))])

            # ---- layer 2 block finisher: normalize -> output ----
            def finish2(b, pacc):
                rows = P if b < NBLK - 1 else LASTB
                se = sb.tile([P, 1], F32, tag="se2")
                nc.vector.tensor_scalar_add(out=se[:], in0=pacc[:, REPR:REPR + 1],
                                            scalar1=1e-16)
                rec = sb.tile([P, 1], F32, tag="rec2")
                nc.vector.reciprocal(out=rec[:], in_=se[:])
                o = sb.tile([P, REPR], F32, tag="o")
                nc.scalar.activation(out=o[:], in_=pacc[:, 0:REPR], func=AF.Copy,
                                     scale=rec[:, 0:1])
                if b2_any:
                    nc.vector.tensor_tensor(out=o[:], in0=o[:], in1=b2e[:], op=ALU.add)
                nc.sync.dma_start(out=out_t[b * P:b * P + rows, :], in_=o[:rows, :])

            edge_layer(t2_full, T2_ELEM, REPR, 1, adst2, finish2)

    nc.finalize()
    globals()["LAST_NC"] = nc
    return nc


def kernel(**inputs):
    node_emb = np.asarray(inputs["node_emb"], np.float32)
    W1 = np.asarray(inputs["W1"], np.float32)
    att1_src = np.asarray(inputs["att1_src"], np.float32)
    att1_dst = np.asarray(inputs["att1_dst"], np.float32)
    b1 = np.asarray(inputs["b1"], np.float32)
    W2 = np.asarray(inputs["W2"], np.float32)
    att2_src = np.asarray(inputs["att2_src"], np.float32)
    att2_dst = np.asarray(inputs["att2_dst"], np.float32)
    b2 = np.asarray(inputs["b2"], np.float32)
    edge_index = np.asarray(inputs["edge_index"])

    kchs, half_b, idx_w, dl_T = _prep_edges(edge_index)

    # fold attention projections into the GEMMs: a_src = emb @ (W1 . att)
    A1s = np.einsum("ehd,hd->eh", W1.reshape(EMB, HEADS, OUT1), att1_src)
    A1d = np.einsum("ehd,hd->eh", W1.reshape(EMB, HEADS, OUT1), att1_dst)
    w1ext = np.concatenate([W1, A1s, A1d], axis=1).astype(np.float16)
    A2s = W2 @ att2_src[0]
    A2d = W2 @ att2_dst[0]
    w2ext = np.concatenate([W2, A2s[:, None], A2d[:, None]], axis=1).astype(np.float16)

    iota = np.tile(np.arange(P, dtype=np.float16), (P, 1))
    iotar = np.tile(np.arange(P, dtype=np.float16), (P, sum(kchs)))
    b1_any = bool(np.any(b1))
    b2_any = bool(np.any(b2))

    nc = _build(kchs, half_b, b1_any, b2_any)

    embT_pad = np.zeros((NC, EMB, NBLK * P), np.float16)
    for c in range(NC):
        embT_pad[c, :, :NSH] = node_emb[c * NSH:(c + 1) * NSH].T.astype(np.float16)

    in_maps = []
    for c in range(NC):
        m = {
            "embT": embT_pad[c],
            "w1ext": w1ext,
            "w2ext": w2ext,
            "idxw": idx_w[c],
            "dlT": dl_T[c],
            "iota": iota,
            "iotar": iotar,
        }
        if b1_any:
            m["b1e"] = np.tile(b1[None, :], (P, 1)).astype(np.float16)
        if b2_any:
            m["b2e"] = np.tile(b2[None, :], (P, 1)).astype(np.float16)
        in_maps.append(m)

    res = run_bass_kernel_spmd(nc, in_maps, core_ids=list(range(NC)))
    out = np.concatenate([res.results[c]["out"] for c in range(NC)], axis=0)
    return np.ascontiguousarray(out.astype(np.float32))


if __name__ == "__main__":
    # quick self-exercise with random inputs of the right shapes
    rng = np.random.default_rng(0)
    ins = {
        "node_emb": rng.standard_normal((N, EMB), dtype=np.float32) * 0.05,
        "W1": rng.standard_normal((EMB, HIDDEN), dtype=np.float32) * 0.07,
        "att1_src": rng.standard_normal((HEADS, OUT1), dtype=np.float32) * 0.2,
        "att1_dst": rng.standard_normal((HEADS, OUT1), dtype=np.float32) * 0.2,
        "b1": np.zeros(HIDDEN, np.float32),
        "W2": rng.standard_normal((HIDDEN, REPR), dtype=np.float32) * 0.07,
        "att2_src": rng.standard_normal((1, REPR), dtype=np.float32) * 0.2,
        "att2_dst": rng.standard_normal((1, REPR), dtype=np.float32) * 0.2,
        "b2": np.zeros(REPR, np.float32),
        "edge_index": rng.integers(0, N, (2, E)).astype(np.int32),
    }
    out = kernel(**ins)
    print("out", out.shape, out.dtype, np.abs(out).mean())
